# revision 26
# baseline (speedup 1.0000x reference)
"""Trainium2 Bass kernel for AttentiveSSMNoProjCyc (sparse_attention).

Sharding: 8 cores = 2 batches x 4 head-groups (4 heads / 256 channels each).
Per core, [channel, time] layout, bf16 datapath (tolerance is 2e-2):
  - SSM scans via tensor_tensor_scan (DVE, f32 A-tables from host / bf16 x);
    (1-a)*h scaling on Act, residual add on DVE in bf16
  - RoPE never applied to full sequences; rotation algebra instead:
      band s=t   : q.k unrotated (R(t)^T R(t) = I)
      band s=t-1 : k' = R(-1) k via a constant block-diag PE matmul
      boundary   : score = (q*cos_t).u + (q*sin_t).u~ with u = R(s_b) k_b
                   rotated cheaply on 48 columns
  - head score reduction via 64-block-ones matmul, which also replicates
    scores/denominators across each head's 64 channel partitions
  - Pool (gpsimd) engine: SWDGE input DMAs, mask muls, big bf16 adds
  - all matmuls bf16 (4x PE rate); inputs/outputs bf16 (half DMA traffic)
Host: slice/transpose/pack tables; sum 4 bf16 partials per batch in f32.
"""
import numpy as np
import ml_dtypes

import concourse.bass as bass
import concourse.mybir as mybir
from concourse.bass_utils import run_bass_kernel_spmd
from concourse.tile import TileContext
import concourse.tile as _tile_mod
from concourse.vector_clock import ScopedClock as _ScopedClock


def _split_drain_and_barrier(self, tick_clock, wait_clock):
    """Tail drain, with its sem waits spread over chained SP nops.

    Walrus's TPB_CTRL lowering only accepts a couple of sync waits per
    instruction; redistribute them one-per-nop (same engine, program
    order => semantics preserved).
    """
    probe = self.nc.sync.nop()
    wait_clock.add_sem_waits(
        probe.ins, _ScopedClock({None: tick_clock.global_clock})
    )
    si = probe.ins.sync_info
    waits = list(si.on_wait) if si is not None else []
    upds = list(si.on_update) if si is not None else []
    MAXW = 1
    if len(waits) > MAXW:
        probe.ins.sync_info = mybir.SyncInfo(on_wait=waits[:MAXW],
                                             on_update=upds)
        for i in range(MAXW, len(waits), MAXW):
            extra = self.nc.sync.nop()
            extra.ins.sync_info = mybir.SyncInfo(
                on_wait=waits[i:i + MAXW], on_update=[])
    self.nc.sync.drain()

    self.nc.all_engine_barrier()
    assert self.sems is not None
    popped = self.nc._tile_sem_poison_stack.pop()
    assert popped is self._sem_poison
    self.nc.clear_and_free_semaphores(list(self.sems.allocated().values()))
    self.nc.all_engine_barrier()


_tile_mod.TileContext._drain_and_barrier = _split_drain_and_barrier


def _cap_sync_waits(nc, cap=1):
    """Hoist excess sync waits onto same-engine carrier NOPs (walrus only
    accepts `cap` waits per instruction)."""
    nid = [0]

    def mknop(engine, waits):
        nid[0] += 1
        nop = mybir.InstNoOp(name=f"I-capw-{nid[0]}", ins=[], outs=[])
        nop.engine = engine
        nop.sync_info = mybir.SyncInfo(on_wait=list(waits), on_update=[])
        return nop

    for bb in nc.m.functions[0].blocks:
        il = bb.instructions
        i = 0
        while i < len(il):
            ins = il[i]
            si = ins.sync_info
            nw = len(si.on_wait) if si is not None else 0
            if nw > cap:
                waits = list(si.on_wait)
                ins.sync_info = mybir.SyncInfo(on_wait=waits[:cap],
                                               on_update=list(si.on_update))
                rest = waits[cap:]
                pos = i
                for j in range(0, len(rest), cap):
                    il.insert(pos, mknop(ins.engine, rest[j:j + cap]))
                    pos += 1
                    i += 1
            i += 1


B, S, D, H, HD = 2, 2048, 1024, 16, 64
NG = 4            # head-groups per batch
CH = 256          # channels per core (4 heads)
NB = 48           # padded boundary columns (33 real)
NBD = 112         # blockdiag boundary cols: head0 -> 0:48, head1 -> 64:112
NCHUNK = 4
CS = S // NCHUNK  # 512
F32 = mybir.dt.float32
BF16 = mybir.dt.bfloat16
AL = mybir.AluOpType
AF = mybir.ActivationFunctionType
NEG = -1e30
BF = ml_dtypes.bfloat16


def _boundaries():
    K_, LAYER_, NLAYERS_, MAXLEN_ = 64, 4, 16, 16384
    off = min(K_ - 1, LAYER_ * (K_ // NLAYERS_))
    bl = [b - off for b in range(K_ - 1, MAXLEN_, K_)]
    if bl[-1] != MAXLEN_ - 1:
        bl.append(MAXLEN_ - 1)
    if bl[0] != 0:
        bl.insert(0, 0)
    b = np.asarray(bl)
    b = b[b < S].copy()
    b[-1] = S - 1
    return b


BND = _boundaries()
NBR = len(BND)  # 33

SHUF_XOR1 = [i ^ 1 for i in range(32)]


def build_program():
    nc = bass.Bass()
    dp = nc.declare_dram_parameter
    xt16 = dp("xt16", [D, S], BF16, isOutput=False)
    wqp = dp("wqp", [128, 8 * CH], BF16, isOutput=False)
    wop = dp("wop", [128, 2 * D], BF16, isOutput=False)
    nrs = dp("nrs", [128, S], BF16, isOutput=False)
    ctab = dp("ctab", [128, S], BF16, isOutput=False)
    stab = dp("stab", [128, S], BF16, isOutput=False)
    maskb = dp("maskb", [NBD, S], BF16, isOutput=False)
    scp = dp("scp", [128, 233], F32, isOutput=False)
    bcp = dp("bcp", [128, 512], BF16, isOutput=False)
    outp = dp("outp", [D, S], BF16, isOutput=True)

    HS = S // 2           # 1024: half width for chunk pairs
    SEG = 1072            # reset-aligned scan split (48 + 64*16)

    with TileContext(nc) as tc:
        with (
            tc.tile_pool(name="persist", bufs=1) as pp,
            tc.tile_pool(name="xbig", bufs=1) as xb,
            tc.tile_pool(name="atab", bufs=2) as ap2,
            tc.tile_pool(name="sc8k", bufs=2) as sc,
            tc.tile_pool(name="hs16", bufs=2) as hsp,
            tc.tile_pool(name="px", bufs=4) as px,
            tc.tile_pool(name="wk", bufs=2) as wk,        # per-half work tiles
            tc.tile_pool(name="small", bufs=1) as ck,
            tc.tile_pool(name="psA", bufs=5, space="PSUM") as psA,
            tc.tile_pool(name="psB", bufs=3, space="PSUM") as psB,
            nc.allow_low_precision(reason="bf16 datapath; tol 2e-2"),
        ):
            # ============ input DMAs ============
            scp_t = pp.tile([128, 233], F32, tag="scp", name="scp_t")
            nc.sync.dma_start(out=scp_t, in_=scp[:, :])
            x_t = [xb.tile([128, S], BF16, tag=f"xt{k}", name=f"x_t{k}")
                   for k in range(8)]
            nrs_t = pp.tile([128, S], BF16, tag="nrs", name="nrs_t")
            nc.sync.dma_start(out=nrs_t, in_=nrs[:, :])
            nc.sync.dma_start(out=x_t[0], in_=xt16[0:128, :])
            nc.sync.dma_start(out=x_t[1], in_=xt16[128:256, :])
            nc.sync.dma_start(out=x_t[2], in_=xt16[256:384, :])
            nc.sync.dma_start(out=x_t[3], in_=xt16[384:512, :])
            bcp_t = pp.tile([128, 512], BF16, tag="bcp", name="bcp_t")
            ctab_t = pp.tile([128, S], BF16, tag="ctab", name="ctab_t")
            stab_t = pp.tile([128, S], BF16, tag="stab", name="stab_t")
            maskb_t = pp.tile([NBD, S], BF16, tag="maskb", name="maskb_t")
            wqp_t = pp.tile([128, 8 * CH], BF16, tag="wqp", name="wqp_t")
            wop_t = pp.tile([128, 2 * D], BF16, tag="wop", name="wop_t")
            nc.gpsimd.dma_start(out=wqp_t, in_=wqp[:, :])
            for k in range(4, 8):
                nc.gpsimd.dma_start(out=x_t[k],
                                    in_=xt16[k * 128:(k + 1) * 128, :])
            # A_v tables on Pool while Act builds A_k
            av_t = [ap2.tile([128, S], F32, tag="avtile", name=f"Av{dt}")
                    for dt in range(2)]
            for dt in range(2):
                nc.gpsimd.tensor_scalar(out=av_t[dt], in0=nrs_t,
                                        scalar1=scp_t[:, 103 + dt:104 + dt],
                                        scalar2=None, op0=AL.mult)
            nc.gpsimd.dma_start(out=ctab_t, in_=ctab[:, :])
            nc.gpsimd.dma_start(out=stab_t, in_=stab[:, :])
            nc.gpsimd.dma_start(out=bcp_t, in_=bcp[:, :])
            nc.gpsimd.dma_start(out=maskb_t, in_=maskb[:, :])
            nc.gpsimd.dma_start(out=wop_t, in_=wop[:, :])

            cbb = scp_t[:, 0:48]
            sbb = scp_t[:, 48:96]
            pmv = scp_t[:, 96:97]
            omap = scp_t[:, 97:101]
            asig = scp_t[:, 101:105]
            ident32 = scp_t[:, 105:233]
            O128 = bcp_t[:, 0:128]
            obv128 = bcp_t[:, 128:256]
            rotm = bcp_t[:, 256:384]

            # ============ PE p-state warmup ============
            warm = psA.tile([128, CS], F32, tag="psa", name="warm")
            for i in range(5):
                nc.tensor.matmul(warm[:, 0:233], scp_t[:, 0:128], scp_t,
                                 start=(i == 0), stop=(i == 4))
            wsink = ck.tile([128, 8], F32, tag="wsink", name="wsink")
            nc.scalar.activation(wsink, warm[:, 0:8], AF.Copy)

            # ============ persistent tiles ============
            kpre = [pp.tile([128, S], BF16, tag=f"kpre{dt}", name=f"kpre{dt}")
                    for dt in range(2)]
            v16 = [pp.tile([128, S], BF16, tag=f"v16{dt}", name=f"v16{dt}")
                   for dt in range(2)]
            xq = [px.tile([128, S], BF16, tag="px", name=f"xq{dt}")
                  for dt in range(2)]
            kp = [px.tile([128, S], BF16, tag="px", name=f"kp{dt}")
                  for dt in range(2)]
            ats = {}
            for dt in range(2):
                A_t = ap2.tile([128, S], F32, tag="atile", name=f"A{dt}")
                nc.scalar.activation(A_t, nrs_t, AF.Copy,
                                     scale=asig[:, dt:dt + 1])
                ats[dt] = A_t
            # boundary persistents (zeroed once; filled per half)
            kb = [ck.tile([128, NB], BF16, tag=f"kb{dt}", name=f"kb{dt}")
                  for dt in range(2)]
            vb = [ck.tile([128, NB], F32, tag=f"vb{dt}", name=f"vb{dt}")
                  for dt in range(2)]
            kbdA = [ck.tile([128, NBD], BF16, tag=f"kA{dt}", name=f"kbdA{dt}")
                    for dt in range(2)]
            kbdB = [ck.tile([128, NBD], BF16, tag=f"kB{dt}", name=f"kbdB{dt}")
                    for dt in range(2)]
            vbT = [pp.tile([128, 64], BF16, tag=f"vbT{dt}", name=f"vbT{dt}")
                   for dt in range(2)]
            for dt in range(2):
                nc.vector.memset(kb[dt], 0.0)
                nc.vector.memset(vb[dt], 0.0)
                nc.vector.memset(kbdA[dt], 0.0)
                nc.vector.memset(kbdB[dt], 0.0)
                nc.vector.memset(vbT[dt], 0.0)

            # ============ two-half wavefront ============
            for half in range(2):
                lo, hi = half * HS, (half + 1) * HS
                hsl = slice(lo, hi)
                cs = (2 * half, 2 * half + 1)
                ssl = slice(0, SEG) if half == 0 else slice(SEG, S)
                w = ssl.stop - ssl.start

                # ---- scans + hs + residual ----
                for par, outs in enumerate((kpre, v16)):
                    for dt in range(2):
                        col = 2 * par + dt
                        A_t = ats[dt] if par == 0 else av_t[dt]
                        h_t = sc.tile([128, SEG], F32, tag="sc8k",
                                      name=f"h{col}_{half}")
                        nc.vector.tensor_tensor_scan(
                            out=h_t[:, 0:w], data0=A_t[:, ssl],
                            data1=x_t[dt][:, ssl], initial=0.0,
                            op0=AL.mult, op1=AL.add)
                        hst = hsp.tile([128, SEG], BF16, tag="hs16",
                                       name=f"hs{col}_{half}")
                        nc.scalar.activation(hst[:, 0:w], h_t[:, 0:w],
                                             AF.Copy,
                                             scale=omap[:, col:col + 1])
                        nc.vector.tensor_add(out=outs[dt][:, ssl],
                                             in0=hst[:, 0:w],
                                             in1=x_t[dt][:, ssl])

                # ---- Q projection ----
                for m in range(2):
                    accs = {c: psA.tile([128, CS], F32, tag="psa",
                                        name=f"qacc{half}_{m}_{c}")
                            for c in cs}
                    for k in range(8):
                        st_sl = wqp_t[:,
                                      k * CH + m * 128:k * CH + m * 128 + 128]
                        for c in cs:
                            nc.tensor.matmul(
                                accs[c], st_sl,
                                x_t[k][:, c * CS:(c + 1) * CS],
                                start=(k == 0), stop=(k == 7))
                    for c in cs:
                        nc.scalar.activation(xq[m][:, c * CS:(c + 1) * CS],
                                             accs[c], AF.Copy)

                # ---- k' = R(-1) k ----
                for dt in range(2):
                    for c in cs:
                        chs = slice(c * CS, (c + 1) * CS)
                        kps = psB.tile([128, CS], F32, tag="psb",
                                       name=f"kps{dt}_{c}")
                        nc.tensor.matmul(kps, rotm, kpre[dt][:, chs],
                                         start=True, stop=True)
                        nc.scalar.activation(kp[dt][:, chs], kps, AF.Copy)

                # ---- qc/qs (Pool: dt0, DVE: dt1) ----
                qch = {}
                for dt in range(2):
                    qch[('c', dt)] = wk.tile([128, HS], BF16, tag="qch",
                                             name=f"qc{dt}_{half}", bufs=4)
                    qch[('s', dt)] = wk.tile([128, HS], BF16, tag="qch",
                                             name=f"qs{dt}_{half}", bufs=4)
                nc.gpsimd.tensor_tensor(out=qch[('c', 0)],
                                        in0=xq[0][:, hsl],
                                        in1=ctab_t[:, hsl], op=AL.mult)
                nc.gpsimd.tensor_tensor(out=qch[('s', 0)],
                                        in0=xq[0][:, hsl],
                                        in1=stab_t[:, hsl], op=AL.mult)
                nc.vector.tensor_mul(out=qch[('c', 1)], in0=xq[1][:, hsl],
                                     in1=ctab_t[:, hsl])
                nc.vector.tensor_mul(out=qch[('s', 1)], in0=xq[1][:, hsl],
                                     in1=stab_t[:, hsl])

                # ---- band products ----
                prods = {}
                for dt in range(2):
                    p1 = wk.tile([128, HS], BF16, tag="pr16",
                                 name=f"pr1_{dt}_{half}", bufs=4)
                    nc.vector.tensor_mul(out=p1, in0=xq[dt][:, hsl],
                                         in1=kpre[dt][:, hsl])
                    p0 = wk.tile([128, HS], BF16, tag="pr16",
                                 name=f"pr0_{dt}_{half}", bufs=4)
                    if half == 0:
                        nc.vector.memset(p0[:, 0:1], 0.0)
                        nc.vector.tensor_mul(out=p0[:, 1:HS],
                                             in0=xq[dt][:, 1:HS],
                                             in1=kp[dt][:, 0:HS - 1])
                    else:
                        nc.vector.tensor_mul(out=p0,
                                             in0=xq[dt][:, hsl],
                                             in1=kp[dt][:, lo - 1:hi - 1])
                    prods[dt] = (p1, p0)

                # ---- band scores + exps ----
                eh = {}
                for dt in range(2):
                    eh[(1, dt)] = wk.tile([128, HS], BF16, tag="eh",
                                          name=f"e1_{dt}_{half}", bufs=4)
                    eh[(0, dt)] = wk.tile([128, HS], BF16, tag="eh",
                                          name=f"e0_{dt}_{half}", bufs=4)
                for dt in range(2):
                    p1, p0 = prods[dt]
                    for c in cs:
                        rel = slice((c % 2) * CS, (c % 2) * CS + CS)
                        s1p = psB.tile([128, CS], F32, tag="psb",
                                       name=f"s1p{dt}_{c}")
                        nc.tensor.matmul(s1p, O128, p1[:, rel],
                                         start=True, stop=True)
                        nc.scalar.activation(eh[(1, dt)][:, rel], s1p,
                                             AF.Exp, scale=0.125)
                        s0p = psB.tile([128, CS], F32, tag="psb",
                                       name=f"s0p{dt}_{c}")
                        nc.tensor.matmul(s0p, O128, p0[:, rel],
                                         start=True, stop=True)
                        if c == 0:
                            nc.vector.memset(s0p[:, 0:1], NEG)
                        nc.scalar.activation(eh[(0, dt)][:, rel], s0p,
                                             AF.Exp, scale=0.125)

                # ---- boundary keys for this half ----
                # col j of kb/vb maps to t: j=0 -> 0, 1<=j<=31 -> 64(j-1)+47,
                # j=32 -> 2047.  half0 covers j 0..17, half1 j 18..32.
                for dt in range(2):
                    if half == 0:
                        jsl = slice(0, 18)
                        for src_t, dst_t in ((kpre[dt], kb[dt]),
                                             (v16[dt], vb[dt])):
                            nc.vector.tensor_copy(out=dst_t[:, 0:1],
                                                  in_=src_t[:, 0:1])
                            nc.vector.tensor_copy(
                                out=dst_t[:, 1:18],
                                in_=src_t.rearrange("p (a b) -> p a b",
                                                    b=64)[:, 0:17, 47])
                    else:
                        jsl = slice(18, 33)
                        for src_t, dst_t in ((kpre[dt], kb[dt]),
                                             (v16[dt], vb[dt])):
                            nc.vector.tensor_copy(
                                out=dst_t[:, 18:32],
                                in_=src_t.rearrange("p (a b) -> p a b",
                                                    b=64)[:, 17:31, 47])
                            nc.vector.tensor_copy(out=dst_t[:, 32:33],
                                                  in_=src_t[:, S - 1:S])
                    jw = jsl.stop - jsl.start
                    kbsh = ck.tile([128, NB], BF16, tag="kbs",
                                   name=f"kbsh{dt}_{half}")
                    nc.vector.stream_shuffle(kbsh[:, jsl], kb[dt][:, jsl],
                                             SHUF_XOR1)
                    t1 = ck.tile([128, NB], BF16, tag="kbt",
                                 name=f"t1_{dt}_{half}")
                    nc.vector.tensor_mul(out=t1[:, jsl], in0=kb[dt][:, jsl],
                                         in1=cbb[:, jsl])
                    nc.vector.tensor_mul(out=kbsh[:, jsl],
                                         in0=kbsh[:, jsl], in1=sbb[:, jsl])
                    u16 = ck.tile([128, NB], BF16, tag="kbv",
                                  name=f"u16_{dt}_{half}")
                    nc.vector.tensor_add(out=u16[:, jsl], in0=t1[:, jsl],
                                         in1=kbsh[:, jsl])
                    ush = ck.tile([128, NB], BF16, tag="kbw",
                                  name=f"ush{dt}_{half}")
                    nc.vector.stream_shuffle(ush[:, jsl], u16[:, jsl],
                                             SHUF_XOR1)
                    nc.vector.tensor_scalar(out=ush[:, jsl],
                                            in0=ush[:, jsl], scalar1=pmv,
                                            scalar2=None, op0=AL.mult)
                    for src_t, dst_t in ((u16, kbdA[dt]), (ush, kbdB[dt])):
                        nc.vector.tensor_copy(out=dst_t[0:64, jsl],
                                              in_=src_t[0:64, jsl])
                        nc.vector.tensor_copy(
                            out=dst_t[64:128, 64 + jsl.start:64 + jsl.stop],
                            in_=src_t[64:128, jsl])
                    # partition writes must start 64-aligned: half1 redoes
                    # cols 0:33 so the vbT write starts at hh*64
                    tjsl = jsl if half == 0 else slice(0, 33)
                    tw = tjsl.stop - tjsl.start
                    for hh in range(2):
                        tp = psB.tile([128, CS], F32, tag="psb",
                                      name=f"tp{dt}_{hh}_{half}")
                        nc.tensor.transpose(
                            tp[0:tw, 0:64],
                            vb[dt][hh * 64:(hh + 1) * 64, tjsl],
                            ident32[hh * 64:(hh + 1) * 64,
                                    hh * 64:(hh + 1) * 64],
                            tile_position=(hh * 64, 0))
                        nc.scalar.activation(
                            vbT[dt][hh * 64:hh * 64 + tw, :],
                            tp[0:tw, 0:64], AF.Copy)

                # ---- boundary scores ----
                embdh = {}
                for dt in range(2):
                    emb = wk.tile([128, HS], BF16, tag="embdh",
                                  name=f"embd{dt}_{half}", bufs=2)
                    embdh[dt] = emb
                    for c in cs:
                        rel = slice((c % 2) * CS, (c % 2) * CS + CS)
                        chs = slice(c * CS, (c + 1) * CS)
                        eb = psB.tile([128, CS], F32, tag="psb",
                                      name=f"eb{dt}_{c}")
                        nc.tensor.matmul(eb[0:NBD, :], kbdA[dt],
                                         qch[('c', dt)][:, rel],
                                         start=True, stop=False)
                        nc.tensor.matmul(eb[0:NBD, :], kbdB[dt],
                                         qch[('s', dt)][:, rel],
                                         start=False, stop=True)
                        nc.scalar.activation(emb[0:NBD, rel], eb[0:NBD, :],
                                             AF.Exp, scale=0.125)
                        nc.gpsimd.tensor_tensor(out=emb[0:NBD, rel],
                                                in0=emb[0:NBD, rel],
                                                in1=maskb_t[:, chs],
                                                op=AL.mult)

                # ---- denominators ----
                rdh = {}
                for dt in range(2):
                    denE = wk.tile([128, HS], BF16, tag="denEh",
                                   name=f"denE{dt}_{half}", bufs=2)
                    nc.gpsimd.tensor_tensor(out=denE, in0=eh[(1, dt)],
                                            in1=eh[(0, dt)], op=AL.add)
                    den = wk.tile([128, HS], BF16, tag="denh",
                                  name=f"den{dt}_{half}", bufs=2)
                    for c in cs:
                        rel = slice((c % 2) * CS, (c % 2) * CS + CS)
                        bs = psB.tile([128, CS], F32, tag="psb",
                                      name=f"bs{dt}_{c}")
                        nc.tensor.matmul(bs, obv128[0:NBD, :],
                                         embdh[dt][0:NBD, rel],
                                         start=True, stop=True)
                        nc.vector.tensor_add(out=den[:, rel],
                                             in0=denE[:, rel], in1=bs)
                    rdh[dt] = wk.tile([128, HS], BF16, tag="rdh",
                                      name=f"rd{dt}_{half}", bufs=2)
                    nc.vector.reciprocal(rdh[dt], den)

                # ---- combine (in-place accumulate chain) ----
                acc16 = {}
                for dt in range(2):
                    acc = wk.tile([128, HS], BF16, tag="acc16",
                                  name=f"acc{dt}_{half}", bufs=3)
                    acc16[dt] = acc
                    nc.vector.tensor_mul(out=acc, in0=eh[(1, dt)],
                                         in1=v16[dt][:, hsl])
                    n2 = wk.tile([128, HS], BF16, tag="n2h",
                                 name=f"n2_{dt}_{half}", bufs=2)
                    if half == 0:
                        nc.vector.memset(n2[:, 0:1], 0.0)
                        nc.vector.tensor_mul(out=n2[:, 1:HS],
                                             in0=eh[(0, dt)][:, 1:HS],
                                             in1=v16[dt][:, 0:HS - 1])
                    else:
                        nc.vector.tensor_mul(out=n2, in0=eh[(0, dt)],
                                             in1=v16[dt][:, lo - 1:hi - 1])
                    nc.gpsimd.tensor_tensor(out=acc, in0=acc, in1=n2,
                                            op=AL.add)
                for dt in range(2):
                    acc = acc16[dt]
                    for c in cs:
                        rel = slice((c % 2) * CS, (c % 2) * CS + CS)
                        pv = psA.tile([128, CS], F32, tag="psa",
                                      name=f"pv{dt}_{c}")
                        for hh in range(2):
                            nc.tensor.matmul(
                                pv[hh * 64:(hh + 1) * 64, :],
                                vbT[dt][hh * 64:hh * 64 + 48, :],
                                embdh[dt][hh * 64:hh * 64 + 48, rel],
                                start=True, stop=True,
                                tile_position=(hh * 64, hh * 64))
                        nc.vector.tensor_add(out=acc[:, rel],
                                             in0=acc[:, rel], in1=pv)
                        nc.vector.tensor_mul(out=acc[:, rel],
                                             in0=acc[:, rel],
                                             in1=rdh[dt][:, rel])

                # ---- output projection + DMA for this half ----
                for m in range(8):
                    stage = wk.tile([128, HS], BF16, tag="stg",
                                    name=f"stage{m}_{half}", bufs=4)
                    for c in cs:
                        rel = slice((c % 2) * CS, (c % 2) * CS + CS)
                        oacc = psA.tile([128, CS], F32, tag="psa",
                                        name=f"oacc{m}_{c}")
                        for k in range(2):
                            st_sl = wop_t[:, k * D + m * 128:
                                          k * D + (m + 1) * 128]
                            nc.tensor.matmul(oacc, st_sl,
                                             acc16[k][:, rel],
                                             start=(k == 0), stop=(k == 1))
                        if c % 4 == 1:
                            nc.vector.tensor_copy(out=stage[:, rel],
                                                  in_=oacc)
                        else:
                            nc.scalar.activation(stage[:, rel], oacc,
                                                 AF.Copy)
                    nc.sync.dma_start(out=outp[m * 128:(m + 1) * 128, hsl],
                                      in_=stage)
    _cap_sync_waits(nc)
    return nc


# ---------------- host side ----------------

def _host_consts(fc):
    # per-partition pair index: i(p) = (p % 64) // 2
    pidx = (np.arange(128) % 64) // 2
    cos_t = fc[:, :, 0, 0]          # (S, 32)
    sin_t = fc[:, :, 1, 0]          # (S, 32)
    ctab = np.ascontiguousarray(cos_t[:, pidx].T).astype(BF)   # (128, S)
    stabt = np.ascontiguousarray(sin_t[:, pidx].T).astype(BF)

    nrs1 = np.ones(S, np.float32)
    starts = np.concatenate([[0], BND[:-1] + 1])
    nrs1[starts] = 0.0

    mb = np.zeros((NBD, S), np.float32)
    t = np.arange(S)
    for hh in range(2):
        for jb in range(NBR):
            mb[hh * 64 + jb] = (t >= BND[jb] + 2).astype(np.float32)
    mb16 = mb.astype(BF)

    cbb = np.zeros((128, NB), np.float32)
    sbb = np.zeros((128, NB), np.float32)
    even = (np.arange(128) % 2 == 0)
    for jb in range(NBR):
        cb = cos_t[BND[jb]][pidx]          # (128,)
        sb = sin_t[BND[jb]][pidx]
        cbb[:, jb] = cb
        sbb[:, jb] = np.where(even, -sb, sb)
    pm = np.where(even, 1.0, -1.0).astype(np.float32)

    # bcp pack: O128 | obv128 | rotm | spare  (bf16)
    O128 = np.zeros((128, 128), np.float32)
    O128[0:64, 0:64] = 1.0
    O128[64:128, 64:128] = 1.0
    obv = np.zeros((128, 128), np.float32)
    obv[0:48, 0:64] = 1.0
    obv[64:112, 64:128] = 1.0
    rotm = np.zeros((128, 128), np.float32)
    c1 = cos_t[1][pidx]   # (128,) per-partition cos(theta_i)
    s1 = sin_t[1][pidx]
    for j in range(64):
        pe_, po = 2 * j, 2 * j + 1
        # kp[2i] = c1*ke + s1*ko ; kp[2i+1] = -s1*ke + c1*ko
        rotm[pe_, pe_] = c1[pe_]
        rotm[po, pe_] = s1[pe_]
        rotm[pe_, po] = -s1[pe_]
        rotm[po, po] = c1[pe_]
    bcp = np.concatenate([O128, obv, rotm,
                          np.zeros((128, 128), np.float32)],
                         axis=1).astype(BF)

    return ctab, stabt, nrs1, mb16, cbb, sbb, pm, bcp


_prog = None


def make_in_maps(x, fc, wq_, wo_, a_k_, a_v_):
    ctab, stabt, nrs1, mb16, cbb, sbb, pm, bcp = _host_consts(fc)
    x16 = x.astype(BF)
    ident = np.eye(128, dtype=np.float32)
    in_maps, metas = [], []
    for b in range(B):
        xT = np.ascontiguousarray(x16[b].T)
        for g in range(NG):
            c0 = g * CH
            perm = np.concatenate([np.arange(c0, c0 + CH),
                                   np.arange(0, c0),
                                   np.arange(c0 + CH, D)]).astype(np.int64)
            xt_core = np.ascontiguousarray(xT[perm])
            wqt = wq_[c0:c0 + CH, :].T[perm]            # (1024, 256)
            wqpk = np.ascontiguousarray(
                wqt.reshape(8, 128, CH).transpose(1, 0, 2).reshape(
                    128, 8 * CH)).astype(BF)
            wot = wo_[:, c0:c0 + CH].T                  # (256, 1024)
            wopk = np.ascontiguousarray(
                wot.reshape(2, 128, D).transpose(1, 0, 2).reshape(
                    128, 2 * D)).astype(BF)
            a_k = 1.0 / (1.0 + np.exp(-a_k_[c0:c0 + CH]))   # sigmoid
            a_v = 1.0 / (1.0 + np.exp(-a_v_[c0:c0 + CH]))
            omap = np.stack([1.0 - a_k[0:128], 1.0 - a_k[128:256],
                             1.0 - a_v[0:128], 1.0 - a_v[128:256]],
                            axis=1).astype(np.float32)
            asig = np.stack([a_k[0:128], a_k[128:256],
                             a_v[0:128], a_v[128:256]],
                            axis=1).astype(np.float32)
            scpk = np.concatenate([cbb, sbb, pm[:, None], omap, asig,
                                   ident], axis=1).astype(np.float32)
            in_maps.append({
                "xt16": xt_core, "wqp": wqpk, "wop": wopk,
                "nrs": np.broadcast_to(nrs1, (128, S)).astype(BF).copy(),
                "ctab": ctab, "stab": stabt, "maskb": mb16, "scp": scpk,
                "bcp": bcp,
            })
            metas.append((b, g))
    return in_maps, metas


def kernel(x, freq_cis, wq, wo, a_k, a_v):
    global _prog
    x = np.asarray(x, np.float32)
    fc = np.asarray(freq_cis, np.float32)
    wq_ = np.asarray(wq, np.float32)
    wo_ = np.asarray(wo, np.float32)
    a_k_ = np.asarray(a_k, np.float32)
    a_v_ = np.asarray(a_v, np.float32)
    in_maps, metas = make_in_maps(x, fc, wq_, wo_, a_k_, a_v_)
    if _prog is None:
        _prog = build_program()
    res = run_bass_kernel_spmd(_prog, in_maps, core_ids=list(range(8)))
    out = np.zeros((B, S, D), np.float32)
    for (b, g), r in zip(metas, res.results):
        out[b] += np.asarray(r["outp"], np.float32).T
    return out


if __name__ == "__main__":
    build_program()
    print("program built ok")


# revision 27
# speedup vs baseline: 1.1130x; 1.1130x over previous
"""Trainium2 Bass kernel for AttentiveSSMNoProjCyc (sparse_attention).

Sharding: 8 cores = 2 batches x 4 head-groups (4 heads / 256 channels each).
Per core, [channel, time] layout, bf16 datapath (tolerance is 2e-2):
  - SSM scans via tensor_tensor_scan (DVE, f32 A-tables from host / bf16 x);
    (1-a)*h scaling on Act, residual add on DVE in bf16
  - RoPE never applied to full sequences; rotation algebra instead:
      band s=t   : q.k unrotated (R(t)^T R(t) = I)
      band s=t-1 : k' = R(-1) k via a constant block-diag PE matmul
      boundary   : score = (q*cos_t).u + (q*sin_t).u~ with u = R(s_b) k_b
                   rotated cheaply on 48 columns
  - head score reduction via 64-block-ones matmul, which also replicates
    scores/denominators across each head's 64 channel partitions
  - Pool (gpsimd) engine: SWDGE input DMAs, mask muls, big bf16 adds
  - all matmuls bf16 (4x PE rate); inputs/outputs bf16 (half DMA traffic)
Host: slice/transpose/pack tables; sum 4 bf16 partials per batch in f32.
"""
import numpy as np
import ml_dtypes

import concourse.bass as bass
import concourse.mybir as mybir
from concourse.bass_utils import run_bass_kernel_spmd
from concourse.tile import TileContext
import concourse.tile as _tile_mod
from concourse.vector_clock import ScopedClock as _ScopedClock


def _split_drain_and_barrier(self, tick_clock, wait_clock):
    """Tail drain, with its sem waits spread over chained SP nops.

    Walrus's TPB_CTRL lowering only accepts a couple of sync waits per
    instruction; redistribute them one-per-nop (same engine, program
    order => semantics preserved).
    """
    probe = self.nc.sync.nop()
    wait_clock.add_sem_waits(
        probe.ins, _ScopedClock({None: tick_clock.global_clock})
    )
    si = probe.ins.sync_info
    waits = list(si.on_wait) if si is not None else []
    upds = list(si.on_update) if si is not None else []
    MAXW = 1
    if len(waits) > MAXW:
        probe.ins.sync_info = mybir.SyncInfo(on_wait=waits[:MAXW],
                                             on_update=upds)
        for i in range(MAXW, len(waits), MAXW):
            extra = self.nc.sync.nop()
            extra.ins.sync_info = mybir.SyncInfo(
                on_wait=waits[i:i + MAXW], on_update=[])
    self.nc.sync.drain()

    self.nc.all_engine_barrier()
    assert self.sems is not None
    popped = self.nc._tile_sem_poison_stack.pop()
    assert popped is self._sem_poison
    self.nc.clear_and_free_semaphores(list(self.sems.allocated().values()))
    self.nc.all_engine_barrier()


_tile_mod.TileContext._drain_and_barrier = _split_drain_and_barrier


def _cap_sync_waits(nc, cap=1):
    """Hoist excess sync waits onto same-engine carrier NOPs (walrus only
    accepts `cap` waits per instruction)."""
    nid = [0]

    def mknop(engine, waits):
        nid[0] += 1
        nop = mybir.InstNoOp(name=f"I-capw-{nid[0]}", ins=[], outs=[])
        nop.engine = engine
        nop.sync_info = mybir.SyncInfo(on_wait=list(waits), on_update=[])
        return nop

    for bb in nc.m.functions[0].blocks:
        il = bb.instructions
        i = 0
        while i < len(il):
            ins = il[i]
            si = ins.sync_info
            nw = len(si.on_wait) if si is not None else 0
            if nw > cap:
                waits = list(si.on_wait)
                ins.sync_info = mybir.SyncInfo(on_wait=waits[:cap],
                                               on_update=list(si.on_update))
                rest = waits[cap:]
                pos = i
                for j in range(0, len(rest), cap):
                    il.insert(pos, mknop(ins.engine, rest[j:j + cap]))
                    pos += 1
                    i += 1
            i += 1


B, S, D, H, HD = 2, 2048, 1024, 16, 64
NG = 4            # head-groups per batch
CH = 256          # channels per core (4 heads)
NB = 48           # padded boundary columns (33 real)
NBD = 112         # blockdiag boundary cols: head0 -> 0:48, head1 -> 64:112
NCHUNK = 4
CS = S // NCHUNK  # 512
F32 = mybir.dt.float32
BF16 = mybir.dt.bfloat16
AL = mybir.AluOpType
AF = mybir.ActivationFunctionType
NEG = -1e30
BF = ml_dtypes.bfloat16


def _boundaries():
    K_, LAYER_, NLAYERS_, MAXLEN_ = 64, 4, 16, 16384
    off = min(K_ - 1, LAYER_ * (K_ // NLAYERS_))
    bl = [b - off for b in range(K_ - 1, MAXLEN_, K_)]
    if bl[-1] != MAXLEN_ - 1:
        bl.append(MAXLEN_ - 1)
    if bl[0] != 0:
        bl.insert(0, 0)
    b = np.asarray(bl)
    b = b[b < S].copy()
    b[-1] = S - 1
    return b


BND = _boundaries()
NBR = len(BND)  # 33

SHUF_XOR1 = [i ^ 1 for i in range(32)]


def build_program():
    nc = bass.Bass()
    dp = nc.declare_dram_parameter
    xt16 = dp("xt16", [D, S], BF16, isOutput=False)
    wqp = dp("wqp", [128, 8 * CH], BF16, isOutput=False)
    wop = dp("wop", [128, 2 * D], BF16, isOutput=False)
    nrs = dp("nrs", [128, S], BF16, isOutput=False)
    ctab = dp("ctab", [128, S], BF16, isOutput=False)
    stab = dp("stab", [128, S], BF16, isOutput=False)
    maskb = dp("maskb", [NBD, S], BF16, isOutput=False)
    scp = dp("scp", [128, 233], F32, isOutput=False)
    bcp = dp("bcp", [128, 512], BF16, isOutput=False)
    outp = dp("outp", [D, S], BF16, isOutput=True)

    HS = S // 2           # 1024: half width for chunk pairs
    SEG = 1072            # reset-aligned scan split (48 + 64*16)

    with TileContext(nc) as tc:
        with (
            tc.tile_pool(name="persist", bufs=1) as pp,
            tc.tile_pool(name="xbig", bufs=1) as xb,
            tc.tile_pool(name="atab", bufs=2) as ap2,
            tc.tile_pool(name="sc8k", bufs=2) as sc,
            tc.tile_pool(name="hs16", bufs=2) as hsp,
            tc.tile_pool(name="px", bufs=4) as px,
            tc.tile_pool(name="wk", bufs=2) as wk,        # per-half work tiles
            tc.tile_pool(name="small", bufs=1) as ck,
            tc.tile_pool(name="psA", bufs=5, space="PSUM") as psA,
            tc.tile_pool(name="psB", bufs=3, space="PSUM") as psB,
            nc.allow_low_precision(reason="bf16 datapath; tol 2e-2"),
        ):
            # ============ input DMAs ============
            scp_t = pp.tile([128, 233], F32, tag="scp", name="scp_t")
            nc.sync.dma_start(out=scp_t, in_=scp[:, :])
            x_t = [xb.tile([128, S], BF16, tag=f"xt{k}", name=f"x_t{k}")
                   for k in range(8)]
            nrs_t = pp.tile([128, S], BF16, tag="nrs", name="nrs_t")
            nc.sync.dma_start(out=nrs_t, in_=nrs[:, :])
            nc.sync.dma_start(out=x_t[0], in_=xt16[0:128, :])
            nc.sync.dma_start(out=x_t[1], in_=xt16[128:256, :])
            nc.sync.dma_start(out=x_t[2], in_=xt16[256:384, :])
            nc.sync.dma_start(out=x_t[3], in_=xt16[384:512, :])
            bcp_t = pp.tile([128, 512], BF16, tag="bcp", name="bcp_t")
            ctab_t = pp.tile([128, S], BF16, tag="ctab", name="ctab_t")
            stab_t = pp.tile([128, S], BF16, tag="stab", name="stab_t")
            maskb_t = pp.tile([NBD, S], BF16, tag="maskb", name="maskb_t")
            wqp_t = pp.tile([128, 8 * CH], BF16, tag="wqp", name="wqp_t")
            wop_t = pp.tile([128, 2 * D], BF16, tag="wop", name="wop_t")
            nc.gpsimd.dma_start(out=wqp_t, in_=wqp[:, :])
            for k in range(4, 8):
                nc.gpsimd.dma_start(out=x_t[k],
                                    in_=xt16[k * 128:(k + 1) * 128, :])
            # A_v tables on Pool while Act builds A_k
            av_t = [ap2.tile([128, S], F32, tag="avtile", name=f"Av{dt}")
                    for dt in range(2)]
            for dt in range(2):
                nc.gpsimd.tensor_scalar(out=av_t[dt], in0=nrs_t,
                                        scalar1=scp_t[:, 103 + dt:104 + dt],
                                        scalar2=None, op0=AL.mult)
            nc.gpsimd.dma_start(out=ctab_t, in_=ctab[:, :])
            nc.gpsimd.dma_start(out=stab_t, in_=stab[:, :])
            nc.gpsimd.dma_start(out=bcp_t, in_=bcp[:, :])
            nc.gpsimd.dma_start(out=maskb_t, in_=maskb[:, :])
            nc.gpsimd.dma_start(out=wop_t, in_=wop[:, :])

            cbb = scp_t[:, 0:48]
            sbb = scp_t[:, 48:96]
            pmv = scp_t[:, 96:97]
            omap = scp_t[:, 97:101]
            asig = scp_t[:, 101:105]
            ident32 = scp_t[:, 105:233]
            O128 = bcp_t[:, 0:128]
            obv128 = bcp_t[:, 128:256]
            rotm = bcp_t[:, 256:384]

            # ============ PE p-state warmup ============
            warm = psA.tile([128, CS], F32, tag="psa", name="warm")
            for i in range(5):
                nc.tensor.matmul(warm[:, 0:233], scp_t[:, 0:128], scp_t,
                                 start=(i == 0), stop=(i == 4))
            wsink = ck.tile([128, 8], F32, tag="wsink", name="wsink")
            nc.scalar.activation(wsink, warm[:, 0:8], AF.Copy)

            # ============ persistent tiles ============
            kpre = [pp.tile([128, S], BF16, tag=f"kpre{dt}", name=f"kpre{dt}")
                    for dt in range(2)]
            v16 = [pp.tile([128, S], BF16, tag=f"v16{dt}", name=f"v16{dt}")
                   for dt in range(2)]
            xq = [px.tile([128, S], BF16, tag="px", name=f"xq{dt}")
                  for dt in range(2)]
            kp = [px.tile([128, S], BF16, tag="px", name=f"kp{dt}")
                  for dt in range(2)]
            ats = {}
            for dt in range(2):
                A_t = ap2.tile([128, S], F32, tag="atile", name=f"A{dt}")
                nc.scalar.activation(A_t, nrs_t, AF.Copy,
                                     scale=asig[:, dt:dt + 1])
                ats[dt] = A_t
            # boundary persistents (zeroed once; filled per half)
            kb = [ck.tile([128, NB], BF16, tag=f"kb{dt}", name=f"kb{dt}")
                  for dt in range(2)]
            vb = [ck.tile([128, NB], F32, tag=f"vb{dt}", name=f"vb{dt}")
                  for dt in range(2)]
            kbdA = [ck.tile([128, NBD], BF16, tag=f"kA{dt}", name=f"kbdA{dt}")
                    for dt in range(2)]
            kbdB = [ck.tile([128, NBD], BF16, tag=f"kB{dt}", name=f"kbdB{dt}")
                    for dt in range(2)]
            vbT = [pp.tile([128, 64], BF16, tag=f"vbT{dt}", name=f"vbT{dt}")
                   for dt in range(2)]
            for dt in range(2):
                nc.vector.memset(kb[dt], 0.0)
                nc.vector.memset(vb[dt], 0.0)
                nc.vector.memset(kbdA[dt], 0.0)
                nc.vector.memset(kbdB[dt], 0.0)
                nc.vector.memset(vbT[dt], 0.0)

            # ============ two-half wavefront ============
            for half in range(2):
                lo, hi = half * HS, (half + 1) * HS
                hsl = slice(lo, hi)
                cs = (2 * half, 2 * half + 1)
                ssl = slice(0, SEG) if half == 0 else slice(SEG, S)
                w = ssl.stop - ssl.start

                # ---- scans + hs + residual ----
                for par, outs in enumerate((kpre, v16)):
                    for dt in range(2):
                        col = 2 * par + dt
                        A_t = ats[dt] if par == 0 else av_t[dt]
                        h_t = sc.tile([128, SEG], F32, tag="sc8k",
                                      name=f"h{col}_{half}")
                        nc.vector.tensor_tensor_scan(
                            out=h_t[:, 0:w], data0=A_t[:, ssl],
                            data1=x_t[dt][:, ssl], initial=0.0,
                            op0=AL.mult, op1=AL.add)
                        hst = hsp.tile([128, SEG], BF16, tag="hs16",
                                       name=f"hs{col}_{half}")
                        nc.scalar.activation(hst[:, 0:w], h_t[:, 0:w],
                                             AF.Copy,
                                             scale=omap[:, col:col + 1])
                        nc.vector.tensor_add(out=outs[dt][:, ssl],
                                             in0=hst[:, 0:w],
                                             in1=x_t[dt][:, ssl])

                # ---- Q projection ----
                for m in range(2):
                    accs = {c: psA.tile([128, CS], F32, tag="psa",
                                        name=f"qacc{half}_{m}_{c}")
                            for c in cs}
                    for k in range(8):
                        st_sl = wqp_t[:,
                                      k * CH + m * 128:k * CH + m * 128 + 128]
                        for c in cs:
                            nc.tensor.matmul(
                                accs[c], st_sl,
                                x_t[k][:, c * CS:(c + 1) * CS],
                                start=(k == 0), stop=(k == 7))
                    for c in cs:
                        nc.scalar.activation(xq[m][:, c * CS:(c + 1) * CS],
                                             accs[c], AF.Copy)

                # ---- k' = R(-1) k ----
                for dt in range(2):
                    for c in cs:
                        chs = slice(c * CS, (c + 1) * CS)
                        kps = psB.tile([128, CS], F32, tag="psb",
                                       name=f"kps{dt}_{c}")
                        nc.tensor.matmul(kps, rotm, kpre[dt][:, chs],
                                         start=True, stop=True)
                        nc.scalar.activation(kp[dt][:, chs], kps, AF.Copy)

                # ---- qc/qs (Pool: dt0, DVE: dt1) ----
                qch = {}
                for dt in range(2):
                    qch[('c', dt)] = wk.tile([128, HS], BF16, tag="qch",
                                             name=f"qc{dt}_{half}", bufs=4)
                    qch[('s', dt)] = wk.tile([128, HS], BF16, tag="qch",
                                             name=f"qs{dt}_{half}", bufs=4)
                nc.gpsimd.tensor_tensor(out=qch[('c', 0)],
                                        in0=xq[0][:, hsl],
                                        in1=ctab_t[:, hsl], op=AL.mult)
                nc.gpsimd.tensor_tensor(out=qch[('s', 0)],
                                        in0=xq[0][:, hsl],
                                        in1=stab_t[:, hsl], op=AL.mult)
                nc.vector.tensor_mul(out=qch[('c', 1)], in0=xq[1][:, hsl],
                                     in1=ctab_t[:, hsl])
                nc.vector.tensor_mul(out=qch[('s', 1)], in0=xq[1][:, hsl],
                                     in1=stab_t[:, hsl])

                # ---- band products ----
                prods = {}
                for dt in range(2):
                    p1 = wk.tile([128, HS], BF16, tag="pr16",
                                 name=f"pr1_{dt}_{half}", bufs=4)
                    nc.vector.tensor_mul(out=p1, in0=xq[dt][:, hsl],
                                         in1=kpre[dt][:, hsl])
                    p0 = wk.tile([128, HS], BF16, tag="pr16",
                                 name=f"pr0_{dt}_{half}", bufs=4)
                    if half == 0:
                        nc.vector.memset(p0[:, 0:1], 0.0)
                        nc.vector.tensor_mul(out=p0[:, 1:HS],
                                             in0=xq[dt][:, 1:HS],
                                             in1=kp[dt][:, 0:HS - 1])
                    else:
                        nc.vector.tensor_mul(out=p0,
                                             in0=xq[dt][:, hsl],
                                             in1=kp[dt][:, lo - 1:hi - 1])
                    prods[dt] = (p1, p0)

                # ---- band scores + exps ----
                eh = {}
                for dt in range(2):
                    eh[(1, dt)] = wk.tile([128, HS], BF16, tag="eh",
                                          name=f"e1_{dt}_{half}", bufs=4)
                    eh[(0, dt)] = wk.tile([128, HS], BF16, tag="eh",
                                          name=f"e0_{dt}_{half}", bufs=4)
                for dt in range(2):
                    p1, p0 = prods[dt]
                    for c in cs:
                        rel = slice((c % 2) * CS, (c % 2) * CS + CS)
                        s1p = psB.tile([128, CS], F32, tag="psb",
                                       name=f"s1p{dt}_{c}")
                        nc.tensor.matmul(s1p, O128, p1[:, rel],
                                         start=True, stop=True)
                        nc.scalar.activation(eh[(1, dt)][:, rel], s1p,
                                             AF.Exp, scale=0.125)
                        s0p = psB.tile([128, CS], F32, tag="psb",
                                       name=f"s0p{dt}_{c}")
                        nc.tensor.matmul(s0p, O128, p0[:, rel],
                                         start=True, stop=True)
                        if c == 0:
                            nc.vector.memset(s0p[:, 0:1], NEG)
                        nc.scalar.activation(eh[(0, dt)][:, rel], s0p,
                                             AF.Exp, scale=0.125)

                # ---- boundary keys for this half ----
                # col j of kb/vb maps to t: j=0 -> 0, 1<=j<=31 -> 64(j-1)+47,
                # j=32 -> 2047.  half0 covers j 0..17, half1 j 18..32.
                for dt in range(2):
                    if half == 0:
                        jsl = slice(0, 18)
                        for src_t, dst_t in ((kpre[dt], kb[dt]),
                                             (v16[dt], vb[dt])):
                            nc.vector.tensor_copy(out=dst_t[:, 0:1],
                                                  in_=src_t[:, 0:1])
                            nc.vector.tensor_copy(
                                out=dst_t[:, 1:18],
                                in_=src_t.rearrange("p (a b) -> p a b",
                                                    b=64)[:, 0:17, 47])
                    else:
                        jsl = slice(18, 33)
                        for src_t, dst_t in ((kpre[dt], kb[dt]),
                                             (v16[dt], vb[dt])):
                            nc.vector.tensor_copy(
                                out=dst_t[:, 18:32],
                                in_=src_t.rearrange("p (a b) -> p a b",
                                                    b=64)[:, 17:31, 47])
                            nc.vector.tensor_copy(out=dst_t[:, 32:33],
                                                  in_=src_t[:, S - 1:S])
                    jw = jsl.stop - jsl.start
                    kbsh = ck.tile([128, NB], BF16, tag="kbs",
                                   name=f"kbsh{dt}_{half}")
                    nc.vector.stream_shuffle(kbsh[:, jsl], kb[dt][:, jsl],
                                             SHUF_XOR1)
                    t1 = ck.tile([128, NB], BF16, tag="kbt",
                                 name=f"t1_{dt}_{half}")
                    nc.vector.tensor_mul(out=t1[:, jsl], in0=kb[dt][:, jsl],
                                         in1=cbb[:, jsl])
                    nc.vector.tensor_mul(out=kbsh[:, jsl],
                                         in0=kbsh[:, jsl], in1=sbb[:, jsl])
                    u16 = ck.tile([128, NB], BF16, tag="kbv",
                                  name=f"u16_{dt}_{half}")
                    nc.vector.tensor_add(out=u16[:, jsl], in0=t1[:, jsl],
                                         in1=kbsh[:, jsl])
                    ush = ck.tile([128, NB], BF16, tag="kbw",
                                  name=f"ush{dt}_{half}")
                    nc.vector.stream_shuffle(ush[:, jsl], u16[:, jsl],
                                             SHUF_XOR1)
                    nc.vector.tensor_scalar(out=ush[:, jsl],
                                            in0=ush[:, jsl], scalar1=pmv,
                                            scalar2=None, op0=AL.mult)
                    for src_t, dst_t in ((u16, kbdA[dt]), (ush, kbdB[dt])):
                        nc.vector.tensor_copy(out=dst_t[0:64, jsl],
                                              in_=src_t[0:64, jsl])
                        nc.vector.tensor_copy(
                            out=dst_t[64:128, 64 + jsl.start:64 + jsl.stop],
                            in_=src_t[64:128, jsl])
                    # partition writes must start 64-aligned: half1 redoes
                    # cols 0:33 so the vbT write starts at hh*64
                    tjsl = jsl if half == 0 else slice(0, 33)
                    tw = tjsl.stop - tjsl.start
                    for hh in range(2):
                        tp = psB.tile([128, CS], F32, tag="psb",
                                      name=f"tp{dt}_{hh}_{half}")
                        nc.tensor.transpose(
                            tp[0:tw, 0:64],
                            vb[dt][hh * 64:(hh + 1) * 64, tjsl],
                            ident32[hh * 64:(hh + 1) * 64,
                                    hh * 64:(hh + 1) * 64],
                            tile_position=(hh * 64, 0))
                        nc.scalar.activation(
                            vbT[dt][hh * 64:hh * 64 + tw, :],
                            tp[0:tw, 0:64], AF.Copy)

                # ---- boundary scores ----
                embdh = {}
                for dt in range(2):
                    emb = wk.tile([128, HS], BF16, tag="embdh",
                                  name=f"embd{dt}_{half}", bufs=2)
                    embdh[dt] = emb
                    for c in cs:
                        rel = slice((c % 2) * CS, (c % 2) * CS + CS)
                        chs = slice(c * CS, (c + 1) * CS)
                        eb = psB.tile([128, CS], F32, tag="psb",
                                      name=f"eb{dt}_{c}")
                        nc.tensor.matmul(eb[0:NBD, :], kbdA[dt],
                                         qch[('c', dt)][:, rel],
                                         start=True, stop=False)
                        nc.tensor.matmul(eb[0:NBD, :], kbdB[dt],
                                         qch[('s', dt)][:, rel],
                                         start=False, stop=True)
                        nc.scalar.activation(emb[0:NBD, rel], eb[0:NBD, :],
                                             AF.Exp, scale=0.125)
                        nc.gpsimd.tensor_tensor(out=emb[0:NBD, rel],
                                                in0=emb[0:NBD, rel],
                                                in1=maskb_t[:, chs],
                                                op=AL.mult)

                # ---- denominators ----
                rdh = {}
                for dt in range(2):
                    denE = wk.tile([128, HS], BF16, tag="denEh",
                                   name=f"denE{dt}_{half}", bufs=2)
                    nc.vector.tensor_add(out=denE, in0=eh[(1, dt)],
                                         in1=eh[(0, dt)])
                    den = wk.tile([128, HS], BF16, tag="denh",
                                  name=f"den{dt}_{half}", bufs=2)
                    for c in cs:
                        rel = slice((c % 2) * CS, (c % 2) * CS + CS)
                        bs = psB.tile([128, CS], F32, tag="psb",
                                      name=f"bs{dt}_{c}")
                        nc.tensor.matmul(bs, obv128[0:NBD, :],
                                         embdh[dt][0:NBD, rel],
                                         start=True, stop=True)
                        nc.vector.tensor_add(out=den[:, rel],
                                             in0=denE[:, rel], in1=bs)
                    rdh[dt] = wk.tile([128, HS], BF16, tag="rdh",
                                      name=f"rd{dt}_{half}", bufs=2)
                    nc.vector.reciprocal(rdh[dt], den)

                # ---- combine (in-place accumulate chain) ----
                acc16 = {}
                for dt in range(2):
                    acc = wk.tile([128, HS], BF16, tag="acc16",
                                  name=f"acc{dt}_{half}", bufs=3)
                    acc16[dt] = acc
                    nc.vector.tensor_mul(out=acc, in0=eh[(1, dt)],
                                         in1=v16[dt][:, hsl])
                    n2 = wk.tile([128, HS], BF16, tag="n2h",
                                 name=f"n2_{dt}_{half}", bufs=2)
                    if half == 0:
                        nc.vector.memset(n2[:, 0:1], 0.0)
                        nc.vector.tensor_mul(out=n2[:, 1:HS],
                                             in0=eh[(0, dt)][:, 1:HS],
                                             in1=v16[dt][:, 0:HS - 1])
                    else:
                        nc.vector.tensor_mul(out=n2, in0=eh[(0, dt)],
                                             in1=v16[dt][:, lo - 1:hi - 1])
                    nc.vector.tensor_add(out=acc, in0=acc, in1=n2)
                for dt in range(2):
                    acc = acc16[dt]
                    for c in cs:
                        rel = slice((c % 2) * CS, (c % 2) * CS + CS)
                        pv = psA.tile([128, CS], F32, tag="psa",
                                      name=f"pv{dt}_{c}")
                        for hh in range(2):
                            nc.tensor.matmul(
                                pv[hh * 64:(hh + 1) * 64, :],
                                vbT[dt][hh * 64:hh * 64 + 48, :],
                                embdh[dt][hh * 64:hh * 64 + 48, rel],
                                start=True, stop=True,
                                tile_position=(hh * 64, hh * 64))
                        nc.vector.tensor_add(out=acc[:, rel],
                                             in0=acc[:, rel], in1=pv)
                        nc.vector.tensor_mul(out=acc[:, rel],
                                             in0=acc[:, rel],
                                             in1=rdh[dt][:, rel])

                # ---- output projection + DMA for this half ----
                for m in range(8):
                    stage = wk.tile([128, HS], BF16, tag="stg",
                                    name=f"stage{m}_{half}", bufs=4)
                    for c in cs:
                        rel = slice((c % 2) * CS, (c % 2) * CS + CS)
                        oacc = psA.tile([128, CS], F32, tag="psa",
                                        name=f"oacc{m}_{c}")
                        for k in range(2):
                            st_sl = wop_t[:, k * D + m * 128:
                                          k * D + (m + 1) * 128]
                            nc.tensor.matmul(oacc, st_sl,
                                             acc16[k][:, rel],
                                             start=(k == 0), stop=(k == 1))
                        if c % 4 == 1:
                            nc.vector.tensor_copy(out=stage[:, rel],
                                                  in_=oacc)
                        else:
                            nc.scalar.activation(stage[:, rel], oacc,
                                                 AF.Copy)
                    nc.sync.dma_start(out=outp[m * 128:(m + 1) * 128, hsl],
                                      in_=stage)
    _cap_sync_waits(nc)
    return nc


# ---------------- host side ----------------

def _host_consts(fc):
    # per-partition pair index: i(p) = (p % 64) // 2
    pidx = (np.arange(128) % 64) // 2
    cos_t = fc[:, :, 0, 0]          # (S, 32)
    sin_t = fc[:, :, 1, 0]          # (S, 32)
    ctab = np.ascontiguousarray(cos_t[:, pidx].T).astype(BF)   # (128, S)
    stabt = np.ascontiguousarray(sin_t[:, pidx].T).astype(BF)

    nrs1 = np.ones(S, np.float32)
    starts = np.concatenate([[0], BND[:-1] + 1])
    nrs1[starts] = 0.0

    mb = np.zeros((NBD, S), np.float32)
    t = np.arange(S)
    for hh in range(2):
        for jb in range(NBR):
            mb[hh * 64 + jb] = (t >= BND[jb] + 2).astype(np.float32)
    mb16 = mb.astype(BF)

    cbb = np.zeros((128, NB), np.float32)
    sbb = np.zeros((128, NB), np.float32)
    even = (np.arange(128) % 2 == 0)
    for jb in range(NBR):
        cb = cos_t[BND[jb]][pidx]          # (128,)
        sb = sin_t[BND[jb]][pidx]
        cbb[:, jb] = cb
        sbb[:, jb] = np.where(even, -sb, sb)
    pm = np.where(even, 1.0, -1.0).astype(np.float32)

    # bcp pack: O128 | obv128 | rotm | spare  (bf16)
    O128 = np.zeros((128, 128), np.float32)
    O128[0:64, 0:64] = 1.0
    O128[64:128, 64:128] = 1.0
    obv = np.zeros((128, 128), np.float32)
    obv[0:48, 0:64] = 1.0
    obv[64:112, 64:128] = 1.0
    rotm = np.zeros((128, 128), np.float32)
    c1 = cos_t[1][pidx]   # (128,) per-partition cos(theta_i)
    s1 = sin_t[1][pidx]
    for j in range(64):
        pe_, po = 2 * j, 2 * j + 1
        # kp[2i] = c1*ke + s1*ko ; kp[2i+1] = -s1*ke + c1*ko
        rotm[pe_, pe_] = c1[pe_]
        rotm[po, pe_] = s1[pe_]
        rotm[pe_, po] = -s1[pe_]
        rotm[po, po] = c1[pe_]
    bcp = np.concatenate([O128, obv, rotm,
                          np.zeros((128, 128), np.float32)],
                         axis=1).astype(BF)

    return ctab, stabt, nrs1, mb16, cbb, sbb, pm, bcp


_prog = None


def make_in_maps(x, fc, wq_, wo_, a_k_, a_v_):
    ctab, stabt, nrs1, mb16, cbb, sbb, pm, bcp = _host_consts(fc)
    x16 = x.astype(BF)
    ident = np.eye(128, dtype=np.float32)
    in_maps, metas = [], []
    for b in range(B):
        xT = np.ascontiguousarray(x16[b].T)
        for g in range(NG):
            c0 = g * CH
            perm = np.concatenate([np.arange(c0, c0 + CH),
                                   np.arange(0, c0),
                                   np.arange(c0 + CH, D)]).astype(np.int64)
            xt_core = np.ascontiguousarray(xT[perm])
            wqt = wq_[c0:c0 + CH, :].T[perm]            # (1024, 256)
            wqpk = np.ascontiguousarray(
                wqt.reshape(8, 128, CH).transpose(1, 0, 2).reshape(
                    128, 8 * CH)).astype(BF)
            wot = wo_[:, c0:c0 + CH].T                  # (256, 1024)
            wopk = np.ascontiguousarray(
                wot.reshape(2, 128, D).transpose(1, 0, 2).reshape(
                    128, 2 * D)).astype(BF)
            a_k = 1.0 / (1.0 + np.exp(-a_k_[c0:c0 + CH]))   # sigmoid
            a_v = 1.0 / (1.0 + np.exp(-a_v_[c0:c0 + CH]))
            omap = np.stack([1.0 - a_k[0:128], 1.0 - a_k[128:256],
                             1.0 - a_v[0:128], 1.0 - a_v[128:256]],
                            axis=1).astype(np.float32)
            asig = np.stack([a_k[0:128], a_k[128:256],
                             a_v[0:128], a_v[128:256]],
                            axis=1).astype(np.float32)
            scpk = np.concatenate([cbb, sbb, pm[:, None], omap, asig,
                                   ident], axis=1).astype(np.float32)
            in_maps.append({
                "xt16": xt_core, "wqp": wqpk, "wop": wopk,
                "nrs": np.broadcast_to(nrs1, (128, S)).astype(BF).copy(),
                "ctab": ctab, "stab": stabt, "maskb": mb16, "scp": scpk,
                "bcp": bcp,
            })
            metas.append((b, g))
    return in_maps, metas


def kernel(x, freq_cis, wq, wo, a_k, a_v):
    global _prog
    x = np.asarray(x, np.float32)
    fc = np.asarray(freq_cis, np.float32)
    wq_ = np.asarray(wq, np.float32)
    wo_ = np.asarray(wo, np.float32)
    a_k_ = np.asarray(a_k, np.float32)
    a_v_ = np.asarray(a_v, np.float32)
    in_maps, metas = make_in_maps(x, fc, wq_, wo_, a_k_, a_v_)
    if _prog is None:
        _prog = build_program()
    res = run_bass_kernel_spmd(_prog, in_maps, core_ids=list(range(8)))
    out = np.zeros((B, S, D), np.float32)
    for (b, g), r in zip(metas, res.results):
        out[b] += np.asarray(r["outp"], np.float32).T
    return out


if __name__ == "__main__":
    build_program()
    print("program built ok")


# revision 28
# speedup vs baseline: 1.1955x; 1.0741x over previous
"""Trainium2 Bass kernel for AttentiveSSMNoProjCyc (sparse_attention).

Sharding: 8 cores = 2 batches x 4 head-groups (4 heads / 256 channels each).
Per core, [channel, time] layout, bf16 datapath (tolerance is 2e-2):
  - SSM scans via tensor_tensor_scan (DVE, f32 A-tables from host / bf16 x);
    (1-a)*h scaling on Act, residual add on DVE in bf16
  - RoPE never applied to full sequences; rotation algebra instead:
      band s=t   : q.k unrotated (R(t)^T R(t) = I)
      band s=t-1 : k' = R(-1) k via a constant block-diag PE matmul
      boundary   : score = (q*cos_t).u + (q*sin_t).u~ with u = R(s_b) k_b
                   rotated cheaply on 48 columns
  - head score reduction via 64-block-ones matmul, which also replicates
    scores/denominators across each head's 64 channel partitions
  - Pool (gpsimd) engine: SWDGE input DMAs, mask muls, big bf16 adds
  - all matmuls bf16 (4x PE rate); inputs/outputs bf16 (half DMA traffic)
Host: slice/transpose/pack tables; sum 4 bf16 partials per batch in f32.
"""
import numpy as np
import ml_dtypes

import concourse.bass as bass
import concourse.mybir as mybir
from concourse.bass_utils import run_bass_kernel_spmd
from concourse.tile import TileContext
import concourse.tile as _tile_mod
from concourse.vector_clock import ScopedClock as _ScopedClock


def _split_drain_and_barrier(self, tick_clock, wait_clock):
    """Tail drain, with its sem waits spread over chained SP nops.

    Walrus's TPB_CTRL lowering only accepts a couple of sync waits per
    instruction; redistribute them one-per-nop (same engine, program
    order => semantics preserved).
    """
    probe = self.nc.sync.nop()
    wait_clock.add_sem_waits(
        probe.ins, _ScopedClock({None: tick_clock.global_clock})
    )
    si = probe.ins.sync_info
    waits = list(si.on_wait) if si is not None else []
    upds = list(si.on_update) if si is not None else []
    MAXW = 1
    if len(waits) > MAXW:
        probe.ins.sync_info = mybir.SyncInfo(on_wait=waits[:MAXW],
                                             on_update=upds)
        for i in range(MAXW, len(waits), MAXW):
            extra = self.nc.sync.nop()
            extra.ins.sync_info = mybir.SyncInfo(
                on_wait=waits[i:i + MAXW], on_update=[])
    self.nc.sync.drain()

    self.nc.all_engine_barrier()
    assert self.sems is not None
    popped = self.nc._tile_sem_poison_stack.pop()
    assert popped is self._sem_poison
    self.nc.clear_and_free_semaphores(list(self.sems.allocated().values()))
    self.nc.all_engine_barrier()


_tile_mod.TileContext._drain_and_barrier = _split_drain_and_barrier


def _cap_sync_waits(nc, cap=1):
    """Hoist excess sync waits onto same-engine carrier NOPs (walrus only
    accepts `cap` waits per instruction)."""
    nid = [0]

    def mknop(engine, waits):
        nid[0] += 1
        nop = mybir.InstNoOp(name=f"I-capw-{nid[0]}", ins=[], outs=[])
        nop.engine = engine
        nop.sync_info = mybir.SyncInfo(on_wait=list(waits), on_update=[])
        return nop

    for bb in nc.m.functions[0].blocks:
        il = bb.instructions
        i = 0
        while i < len(il):
            ins = il[i]
            si = ins.sync_info
            nw = len(si.on_wait) if si is not None else 0
            if nw > cap:
                waits = list(si.on_wait)
                ins.sync_info = mybir.SyncInfo(on_wait=waits[:cap],
                                               on_update=list(si.on_update))
                rest = waits[cap:]
                pos = i
                for j in range(0, len(rest), cap):
                    il.insert(pos, mknop(ins.engine, rest[j:j + cap]))
                    pos += 1
                    i += 1
            i += 1


B, S, D, H, HD = 2, 2048, 1024, 16, 64
NG = 4            # head-groups per batch
CH = 256          # channels per core (4 heads)
NB = 48           # padded boundary columns (33 real)
NBD = 112         # blockdiag boundary cols: head0 -> 0:48, head1 -> 64:112
NCHUNK = 4
CS = S // NCHUNK  # 512
F32 = mybir.dt.float32
BF16 = mybir.dt.bfloat16
AL = mybir.AluOpType
AF = mybir.ActivationFunctionType
NEG = -1e30
BF = ml_dtypes.bfloat16


def _boundaries():
    K_, LAYER_, NLAYERS_, MAXLEN_ = 64, 4, 16, 16384
    off = min(K_ - 1, LAYER_ * (K_ // NLAYERS_))
    bl = [b - off for b in range(K_ - 1, MAXLEN_, K_)]
    if bl[-1] != MAXLEN_ - 1:
        bl.append(MAXLEN_ - 1)
    if bl[0] != 0:
        bl.insert(0, 0)
    b = np.asarray(bl)
    b = b[b < S].copy()
    b[-1] = S - 1
    return b


BND = _boundaries()
NBR = len(BND)  # 33

SHUF_XOR1 = [i ^ 1 for i in range(32)]


def build_program():
    nc = bass.Bass()
    dp = nc.declare_dram_parameter
    xt16 = dp("xt16", [D, S], BF16, isOutput=False)
    wqp = dp("wqp", [128, 8 * CH], BF16, isOutput=False)
    wop = dp("wop", [128, 2 * D], BF16, isOutput=False)
    nrs = dp("nrs", [128, S], BF16, isOutput=False)
    ctab = dp("ctab", [128, S], BF16, isOutput=False)
    stab = dp("stab", [128, S], BF16, isOutput=False)
    maskb = dp("maskb", [NBD, S], BF16, isOutput=False)
    scp = dp("scp", [128, 233], F32, isOutput=False)
    bcp = dp("bcp", [128, 512], BF16, isOutput=False)
    outp = dp("outp", [D, S], BF16, isOutput=True)

    with TileContext(nc) as tc:
        with (
            tc.tile_pool(name="persist", bufs=1) as pp,
            tc.tile_pool(name="xbig", bufs=1) as xb,       # 8 x-tiles resident
            tc.tile_pool(name="sc8k", bufs=2) as sc,       # f32 h scratch
            tc.tile_pool(name="hs16", bufs=2) as hsp,      # bf16 scaled h
            tc.tile_pool(name="px", bufs=4) as px,         # xq,kp -> n1,n2,stage
            tc.tile_pool(name="eat", bufs=4) as ea,        # e1,e0 -> attn
            tc.tile_pool(name="mid", bufs=2) as md,        # s12 (+denE tag)
            tc.tile_pool(name="mid2", bufs=2) as md2,      # den -> s3
            tc.tile_pool(name="small", bufs=1) as ck,
            tc.tile_pool(name="psA", bufs=5, space="PSUM") as psA,
            tc.tile_pool(name="psB", bufs=3, space="PSUM") as psB,
            nc.allow_low_precision(reason="bf16 datapath; tol 2e-2"),
        ):
            # ============ input DMAs ============
            # Critical path via SP HWDGE: scp, x0, Ak0, x1, Ak1, Av0, Av1,
            # x2, x3.  Bulk via Pool SWDGE (keeps HWDGE short at startup).
            scp_t = pp.tile([128, 233], F32, tag="scp", name="scp_t")
            nc.sync.dma_start(out=scp_t, in_=scp[:, :])
            x_t = [xb.tile([128, S], BF16, tag=f"xt{k}", name=f"x_t{k}")
                   for k in range(8)]
            nrs_t = pp.tile([128, S], BF16, tag="nrs", name="nrs_t")
            nc.sync.dma_start(out=nrs_t, in_=nrs[:, :])
            nc.sync.dma_start(out=x_t[0], in_=xt16[0:128, :])
            nc.sync.dma_start(out=x_t[1], in_=xt16[128:256, :])
            nc.sync.dma_start(out=x_t[2], in_=xt16[256:384, :])
            nc.sync.dma_start(out=x_t[3], in_=xt16[384:512, :])
            bcp_t = pp.tile([128, 512], BF16, tag="bcp", name="bcp_t")
            ctab_t = pp.tile([128, S], BF16, tag="ctab", name="ctab_t")
            stab_t = pp.tile([128, S], BF16, tag="stab", name="stab_t")
            maskb_t = pp.tile([NBD, S], BF16, tag="maskb", name="maskb_t")
            wqp_t = px.tile([128, 8 * CH], BF16, tag="px", name="wqp_t")
            wop_t = pp.tile([128, 2 * D], BF16, tag="wop", name="wop_t")
            nc.gpsimd.dma_start(out=wqp_t, in_=wqp[:, :])
            for k in range(4, 8):
                nc.gpsimd.dma_start(out=x_t[k],
                                    in_=xt16[k * 128:(k + 1) * 128, :])
            # A_v tables built on Pool while Act handles A_k + h-scales
            av_t = [sc.tile([128, S], F32, tag="avtile", name=f"Av{dt}")
                    for dt in range(2)]
            for dt in range(2):
                nc.gpsimd.tensor_scalar(out=av_t[dt], in0=nrs_t,
                                        scalar1=scp_t[:, 103 + dt:104 + dt],
                                        scalar2=None, op0=AL.mult)
            nc.gpsimd.dma_start(out=ctab_t, in_=ctab[:, :])
            nc.gpsimd.dma_start(out=stab_t, in_=stab[:, :])
            nc.gpsimd.dma_start(out=bcp_t, in_=bcp[:, :])
            nc.gpsimd.dma_start(out=maskb_t, in_=maskb[:, :])
            nc.gpsimd.dma_start(out=wop_t, in_=wop[:, :])

            cbb = scp_t[:, 0:48]
            sbb = scp_t[:, 48:96]
            pmv = scp_t[:, 96:97]
            omap = scp_t[:, 97:101]       # 1 - sigmoid(a), host-computed
            asig = scp_t[:, 101:105]      # sigmoid(a), host-computed
            ident32 = scp_t[:, 105:233]
            O128 = bcp_t[:, 0:128]
            obv128 = bcp_t[:, 128:256]
            rotm = bcp_t[:, 256:384]

            # ============ PE p-state warmup ============
            # The cost model prices matmuls by how long the PE has been
            # continuously busy (3us to full speed).  Burn the ramp on
            # dummy f32 matmuls over scp while the x tiles stream in.
            warm = psA.tile([128, CS], F32, tag="psa", name="warm")
            for i in range(5):
                nc.tensor.matmul(warm[:, 0:233], scp_t[:, 0:128], scp_t,
                                 start=(i == 0), stop=(i == 4))
            wsink = ck.tile([128, 8], F32, tag="wsink", name="wsink")
            nc.scalar.activation(wsink, warm[:, 0:8], AF.Copy)

            # ============ SSM scans (k then v) ============
            kpre = [pp.tile([128, S], BF16, tag=f"kpre{dt}", name=f"kpre{dt}")
                    for dt in range(2)]
            v16 = [pp.tile([128, S], BF16, tag=f"v16{dt}", name=f"v16{dt}")
                   for dt in range(2)]
            SEG = 1072   # reset-aligned split (48 + 64*16)
            ats = {}
            for dt in range(2):
                A_t = sc.tile([128, S], F32, tag="atile", name=f"A{dt}")
                nc.scalar.activation(A_t, nrs_t, AF.Copy,
                                     scale=asig[:, dt:dt + 1])
                ats[dt] = A_t
            # interleave: all half-0 sub-scans first (unblocks chunks 0/1
            # downstream), then half-1.  h and hs scratch per (chain, half).
            for half, (c0, c1) in enumerate(((0, SEG), (SEG, S))):
                sl = slice(c0, c1)
                w = c1 - c0
                for par, outs in enumerate((kpre, v16)):
                    for dt in range(2):
                        col = 2 * par + dt
                        A_t = ats[dt] if par == 0 else av_t[dt]
                        h_t = sc.tile([128, SEG], F32, tag="sc8k",
                                      name=f"h{col}_{half}")
                        nc.vector.tensor_tensor_scan(
                            out=h_t[:, 0:w], data0=A_t[:, sl],
                            data1=x_t[dt][:, sl], initial=0.0,
                            op0=AL.mult, op1=AL.add)
                        hst = hsp.tile([128, SEG], BF16, tag="hs16",
                                       name=f"hs{col}_{half}")
                        nc.scalar.activation(hst[:, 0:w], h_t[:, 0:w],
                                             AF.Copy,
                                             scale=omap[:, col:col + 1])
                        nc.vector.tensor_add(out=outs[dt][:, sl],
                                             in0=hst[:, 0:w],
                                             in1=x_t[dt][:, sl])

            # ============ Q projection (stationary-reuse order) ============
            xq = [px.tile([128, S], BF16, tag="px", name=f"xq{dt}")
                  for dt in range(2)]
            for m in range(2):
                accs = [psA.tile([128, CS], F32, tag="psa",
                                 name=f"qacc{m}_{c}") for c in range(NCHUNK)]
                for k in range(8):
                    st_sl = wqp_t[:, k * CH + m * 128:k * CH + m * 128 + 128]
                    for c in range(NCHUNK):
                        nc.tensor.matmul(
                            accs[c], st_sl,
                            x_t[k][:, c * CS:(c + 1) * CS],
                            start=(k == 0), stop=(k == 7))
                for c in range(NCHUNK):
                    nc.scalar.activation(xq[m][:, c * CS:(c + 1) * CS],
                                         accs[c], AF.Copy)

            # ============ k' = R(-1) k via PE ============
            kp = [px.tile([128, S], BF16, tag="px", name=f"kp{dt}")
                  for dt in range(2)]
            for dt in range(2):
                for c in range(NCHUNK):
                    chs = slice(c * CS, (c + 1) * CS)
                    kps = psB.tile([128, CS], F32, tag="psb",
                                   name=f"kps{dt}_{c}")
                    nc.tensor.matmul(kps, rotm, kpre[dt][:, chs],
                                     start=True, stop=True)
                    nc.scalar.activation(kp[dt][:, chs], kps, AF.Copy)

            # ============ qc/qs + band products ============
            # qc/qs reuse x-tile buffers (x_t[4..7] dead after Q-proj)
            qc = [xb.tile([128, S], BF16, tag=f"xt{4 + dt}", name=f"qc{dt}")
                  for dt in range(2)]
            qs = [xb.tile([128, S], BF16, tag=f"xt{6 + dt}", name=f"qs{dt}")
                  for dt in range(2)]
            prod1 = [sc.tile([128, S], BF16, tag="pr16", name=f"pr1_{dt}")
                     for dt in range(2)]
            prod0 = [sc.tile([128, S], BF16, tag="pr16", name=f"pr0_{dt}")
                     for dt in range(2)]
            nc.gpsimd.tensor_tensor(out=qc[0], in0=xq[0], in1=ctab_t,
                                    op=AL.mult)
            nc.gpsimd.tensor_tensor(out=qs[0], in0=xq[0], in1=stab_t,
                                    op=AL.mult)
            for dt in range(2):
                nc.vector.tensor_mul(out=prod1[dt], in0=xq[dt],
                                     in1=kpre[dt])
                nc.vector.memset(prod0[dt][:, 0:1], 0.0)
                nc.vector.tensor_mul(out=prod0[dt][:, 1:S],
                                     in0=xq[dt][:, 1:S],
                                     in1=kp[dt][:, 0:S - 1])
            nc.vector.tensor_mul(out=qc[1], in0=xq[1], in1=ctab_t)
            nc.vector.tensor_mul(out=qs[1], in0=xq[1], in1=stab_t)

            # band scores (PE + Act)
            e1 = [ea.tile([128, S], BF16, tag="eat", name=f"e1_{dt}")
                  for dt in range(2)]
            e0 = [ea.tile([128, S], BF16, tag="eat", name=f"e0_{dt}")
                  for dt in range(2)]
            for dt in range(2):
                for c in range(NCHUNK):
                    chs = slice(c * CS, (c + 1) * CS)
                    s1p = psB.tile([128, CS], F32, tag="psb",
                                   name=f"s1p{dt}_{c}")
                    nc.tensor.matmul(s1p, O128, prod1[dt][:, chs],
                                     start=True, stop=True)
                    nc.scalar.activation(e1[dt][:, chs], s1p, AF.Exp,
                                         scale=0.125)
                    s0p = psB.tile([128, CS], F32, tag="psb",
                                   name=f"s0p{dt}_{c}")
                    nc.tensor.matmul(s0p, O128, prod0[dt][:, chs],
                                     start=True, stop=True)
                    if c == 0:
                        nc.vector.memset(s0p[:, 0:1], NEG)
                    nc.scalar.activation(e0[dt][:, chs], s0p, AF.Exp,
                                         scale=0.125)

            # ============ boundary keys ============
            embd = [pp.tile([128, S], BF16, tag=f"embd{dt}", name=f"embd{dt}")
                    for dt in range(2)]
            vbT = [pp.tile([128, 64], BF16, tag=f"vbT{dt}", name=f"vbT{dt}")
                   for dt in range(2)]
            for dt in range(2):
                kb = ck.tile([128, NB], BF16, tag="kb", name=f"kb{dt}")
                vb = ck.tile([128, NB], F32, tag="vb", name=f"vb{dt}")
                for src_t, dst_t in ((kpre[dt], kb), (v16[dt], vb)):
                    nc.vector.tensor_copy(out=dst_t[:, 0:1],
                                          in_=src_t[:, 0:1])
                    nc.vector.tensor_copy(
                        out=dst_t[:, 1:32],
                        in_=src_t.rearrange("p (a b) -> p a b",
                                            b=64)[:, 0:31, 47])
                    nc.vector.tensor_copy(out=dst_t[:, 32:33],
                                          in_=src_t[:, S - 1:S])
                    nc.vector.memset(dst_t[:, 33:NB], 0.0)
                # u = R(s_b) k_b : pair rotation with per-col coeffs
                kbsh = ck.tile([128, NB], BF16, tag="kbs", name=f"kbsh{dt}")
                nc.vector.stream_shuffle(kbsh, kb, SHUF_XOR1)
                t1 = ck.tile([128, NB], BF16, tag="kbt", name=f"t1_{dt}")
                nc.vector.tensor_mul(out=t1, in0=kb, in1=cbb)
                t2 = ck.tile([128, NB], BF16, tag="kbu", name=f"t2_{dt}")
                nc.vector.tensor_mul(out=t2, in0=kbsh, in1=sbb)
                u16 = ck.tile([128, NB], BF16, tag="kbv", name=f"u16_{dt}")
                nc.vector.tensor_add(out=u16, in0=t1, in1=t2)
                ush = ck.tile([128, NB], BF16, tag="kbw", name=f"ush{dt}")
                nc.vector.stream_shuffle(ush, u16, SHUF_XOR1)
                usw = ck.tile([128, NB], BF16, tag="kbx", name=f"usw{dt}")
                nc.vector.tensor_scalar(out=usw, in0=ush, scalar1=pmv,
                                        scalar2=None, op0=AL.mult)
                kbdA = ck.tile([128, NBD], BF16, tag="kbdA", name=f"kbdA{dt}")
                kbdB = ck.tile([128, NBD], BF16, tag="kbdB", name=f"kbdB{dt}")
                for src_t, dst_t in ((u16, kbdA), (usw, kbdB)):
                    nc.vector.memset(dst_t, 0.0)
                    nc.vector.tensor_copy(out=dst_t[0:64, 0:48],
                                          in_=src_t[0:64, :])
                    nc.vector.tensor_copy(out=dst_t[64:128, 64:112],
                                          in_=src_t[64:128, :])
                # vbT: transpose boundary values
                for hh in range(2):
                    tp = psB.tile([128, CS], F32, tag="psb",
                                  name=f"tp{dt}_{hh}")
                    nc.tensor.transpose(tp[0:48, 0:64],
                                        vb[hh * 64:(hh + 1) * 64, 0:48],
                                        ident32[hh * 64:(hh + 1) * 64,
                                                hh * 64:(hh + 1) * 64],
                                        tile_position=(hh * 64, 0))
                    nc.scalar.activation(vbT[dt][hh * 64:hh * 64 + 48, :],
                                         tp[0:48, 0:64], AF.Copy)
                for c in range(NCHUNK):
                    chs = slice(c * CS, (c + 1) * CS)
                    eb = psB.tile([128, CS], F32, tag="psb",
                                  name=f"eb{dt}_{c}")
                    nc.tensor.matmul(eb[0:NBD, :], kbdA, qc[dt][:, chs],
                                     start=True, stop=False)
                    nc.tensor.matmul(eb[0:NBD, :], kbdB, qs[dt][:, chs],
                                     start=False, stop=True)
                    nc.scalar.activation(embd[dt][0:NBD, chs], eb[0:NBD, :],
                                         AF.Exp, scale=0.125)
                    # mask multiply on Pool (bf16, SBUF-only)
                    nc.gpsimd.tensor_tensor(out=embd[dt][0:NBD, chs],
                                            in0=embd[dt][0:NBD, chs],
                                            in1=maskb_t[:, chs], op=AL.mult)

            # ============ n1/n2 early (feed Pool s12) ============
            n1 = [px.tile([128, S], BF16, tag="px", name=f"n1_{dt}")
                  for dt in range(2)]
            n2 = [px.tile([128, S], BF16, tag="px", name=f"n2_{dt}")
                  for dt in range(2)]
            s12 = [md.tile([128, S], BF16, tag="mid", name=f"s12_{dt}")
                   for dt in range(2)]
            for half in range(2):
                lo, hi = half * 2 * CS, (half + 1) * 2 * CS
                for dt in range(2):
                    nc.vector.tensor_mul(out=n1[dt][:, lo:hi],
                                         in0=e1[dt][:, lo:hi],
                                         in1=v16[dt][:, lo:hi])
                    if half == 0:
                        nc.vector.memset(n2[dt][:, 0:1], 0.0)
                    nc.vector.tensor_mul(
                        out=n2[dt][:, max(lo, 1):hi],
                        in0=e0[dt][:, max(lo, 1):hi],
                        in1=v16[dt][:, max(lo, 1) - 1:hi - 1])
                    nc.vector.tensor_add(out=s12[dt][:, lo:hi],
                                         in0=n1[dt][:, lo:hi],
                                         in1=n2[dt][:, lo:hi])

            # ============ denominators ============
            rd = [pp.tile([128, S], BF16, tag=f"rd{dt}", name=f"rd{dt}")
                  for dt in range(2)]
            dens = [md2.tile([128, S], BF16, tag="mid2", name=f"den{dt}")
                    for dt in range(2)]
            denEs = [md.tile([128, S], BF16, tag="midE", name=f"denE{dt}",
                             bufs=2) for dt in range(2)]
            for half in range(2):
                hsl = slice(half * 2 * CS, (half + 1) * 2 * CS)
                for dt in range(2):
                    nc.vector.tensor_add(out=denEs[dt][:, hsl],
                                         in0=e1[dt][:, hsl],
                                         in1=e0[dt][:, hsl])
                    for c in (2 * half, 2 * half + 1):
                        chs = slice(c * CS, (c + 1) * CS)
                        bs = psB.tile([128, CS], F32, tag="psb",
                                      name=f"bs{dt}_{c}")
                        nc.tensor.matmul(bs, obv128[0:NBD, :],
                                         embd[dt][0:NBD, chs],
                                         start=True, stop=True)
                        nc.vector.tensor_add(out=dens[dt][:, chs],
                                             in0=denEs[dt][:, chs], in1=bs)
                    nc.vector.reciprocal(rd[dt][:, hsl], dens[dt][:, hsl])

            # ============ PV + combine ============
            attn = [ea.tile([128, S], BF16, tag="eat", name=f"attn{dt}")
                    for dt in range(2)]
            s3 = [md2.tile([128, S], BF16, tag="mid2", name=f"s3_{dt}")
                  for dt in range(2)]
            for c in range(NCHUNK):
                chs = slice(c * CS, (c + 1) * CS)
                for dt in range(2):
                    pv = psA.tile([128, CS], F32, tag="psa",
                                  name=f"pv{dt}_{c}")
                    for hh in range(2):
                        nc.tensor.matmul(
                            pv[hh * 64:(hh + 1) * 64, :],
                            vbT[dt][hh * 64:hh * 64 + 48, :],
                            embd[dt][hh * 64:hh * 64 + 48, chs],
                            start=True, stop=True,
                            tile_position=(hh * 64, hh * 64))
                    nc.vector.tensor_add(out=s3[dt][:, chs],
                                         in0=s12[dt][:, chs], in1=pv)
                    nc.vector.tensor_mul(out=attn[dt][:, chs],
                                         in0=s3[dt][:, chs],
                                         in1=rd[dt][:, chs])

            # ============ output projection ============
            for m in range(8):
                stage = px.tile([128, S], BF16, tag="px", name=f"stage{m}")
                accs = [psA.tile([128, CS], F32, tag="psa",
                                 name=f"oacc{m}_{c}") for c in range(NCHUNK)]
                for c in range(NCHUNK):
                    chs = slice(c * CS, (c + 1) * CS)
                    for k in range(2):
                        st_sl = wop_t[:,
                                      k * D + m * 128:k * D + (m + 1) * 128]
                        nc.tensor.matmul(accs[c], st_sl, attn[k][:, chs],
                                         start=(k == 0), stop=(k == 1))
                    if c == 1:
                        nc.vector.tensor_copy(out=stage[:, chs], in_=accs[c])
                    else:
                        nc.scalar.activation(stage[:, chs], accs[c], AF.Copy)
                nc.sync.dma_start(out=outp[m * 128:(m + 1) * 128, :],
                                  in_=stage)
    _cap_sync_waits(nc)
    return nc


# ---------------- host side ----------------

def _host_consts(fc):
    # per-partition pair index: i(p) = (p % 64) // 2
    pidx = (np.arange(128) % 64) // 2
    cos_t = fc[:, :, 0, 0]          # (S, 32)
    sin_t = fc[:, :, 1, 0]          # (S, 32)
    ctab = np.ascontiguousarray(cos_t[:, pidx].T).astype(BF)   # (128, S)
    stabt = np.ascontiguousarray(sin_t[:, pidx].T).astype(BF)

    nrs1 = np.ones(S, np.float32)
    starts = np.concatenate([[0], BND[:-1] + 1])
    nrs1[starts] = 0.0

    mb = np.zeros((NBD, S), np.float32)
    t = np.arange(S)
    for hh in range(2):
        for jb in range(NBR):
            mb[hh * 64 + jb] = (t >= BND[jb] + 2).astype(np.float32)
    mb16 = mb.astype(BF)

    cbb = np.zeros((128, NB), np.float32)
    sbb = np.zeros((128, NB), np.float32)
    even = (np.arange(128) % 2 == 0)
    for jb in range(NBR):
        cb = cos_t[BND[jb]][pidx]          # (128,)
        sb = sin_t[BND[jb]][pidx]
        cbb[:, jb] = cb
        sbb[:, jb] = np.where(even, -sb, sb)
    pm = np.where(even, 1.0, -1.0).astype(np.float32)

    # bcp pack: O128 | obv128 | rotm | spare  (bf16)
    O128 = np.zeros((128, 128), np.float32)
    O128[0:64, 0:64] = 1.0
    O128[64:128, 64:128] = 1.0
    obv = np.zeros((128, 128), np.float32)
    obv[0:48, 0:64] = 1.0
    obv[64:112, 64:128] = 1.0
    rotm = np.zeros((128, 128), np.float32)
    c1 = cos_t[1][pidx]   # (128,) per-partition cos(theta_i)
    s1 = sin_t[1][pidx]
    for j in range(64):
        pe_, po = 2 * j, 2 * j + 1
        # kp[2i] = c1*ke + s1*ko ; kp[2i+1] = -s1*ke + c1*ko
        rotm[pe_, pe_] = c1[pe_]
        rotm[po, pe_] = s1[pe_]
        rotm[pe_, po] = -s1[pe_]
        rotm[po, po] = c1[pe_]
    bcp = np.concatenate([O128, obv, rotm,
                          np.zeros((128, 128), np.float32)],
                         axis=1).astype(BF)

    return ctab, stabt, nrs1, mb16, cbb, sbb, pm, bcp


_prog = None


def make_in_maps(x, fc, wq_, wo_, a_k_, a_v_):
    ctab, stabt, nrs1, mb16, cbb, sbb, pm, bcp = _host_consts(fc)
    x16 = x.astype(BF)
    ident = np.eye(128, dtype=np.float32)
    in_maps, metas = [], []
    for b in range(B):
        xT = np.ascontiguousarray(x16[b].T)
        for g in range(NG):
            c0 = g * CH
            perm = np.concatenate([np.arange(c0, c0 + CH),
                                   np.arange(0, c0),
                                   np.arange(c0 + CH, D)]).astype(np.int64)
            xt_core = np.ascontiguousarray(xT[perm])
            wqt = wq_[c0:c0 + CH, :].T[perm]            # (1024, 256)
            wqpk = np.ascontiguousarray(
                wqt.reshape(8, 128, CH).transpose(1, 0, 2).reshape(
                    128, 8 * CH)).astype(BF)
            wot = wo_[:, c0:c0 + CH].T                  # (256, 1024)
            wopk = np.ascontiguousarray(
                wot.reshape(2, 128, D).transpose(1, 0, 2).reshape(
                    128, 2 * D)).astype(BF)
            a_k = 1.0 / (1.0 + np.exp(-a_k_[c0:c0 + CH]))   # sigmoid
            a_v = 1.0 / (1.0 + np.exp(-a_v_[c0:c0 + CH]))
            omap = np.stack([1.0 - a_k[0:128], 1.0 - a_k[128:256],
                             1.0 - a_v[0:128], 1.0 - a_v[128:256]],
                            axis=1).astype(np.float32)
            asig = np.stack([a_k[0:128], a_k[128:256],
                             a_v[0:128], a_v[128:256]],
                            axis=1).astype(np.float32)
            scpk = np.concatenate([cbb, sbb, pm[:, None], omap, asig,
                                   ident], axis=1).astype(np.float32)
            in_maps.append({
                "xt16": xt_core, "wqp": wqpk, "wop": wopk,
                "nrs": np.broadcast_to(nrs1, (128, S)).astype(BF).copy(),
                "ctab": ctab, "stab": stabt, "maskb": mb16, "scp": scpk,
                "bcp": bcp,
            })
            metas.append((b, g))
    return in_maps, metas


def kernel(x, freq_cis, wq, wo, a_k, a_v):
    global _prog
    x = np.asarray(x, np.float32)
    fc = np.asarray(freq_cis, np.float32)
    wq_ = np.asarray(wq, np.float32)
    wo_ = np.asarray(wo, np.float32)
    a_k_ = np.asarray(a_k, np.float32)
    a_v_ = np.asarray(a_v, np.float32)
    in_maps, metas = make_in_maps(x, fc, wq_, wo_, a_k_, a_v_)
    if _prog is None:
        _prog = build_program()
    res = run_bass_kernel_spmd(_prog, in_maps, core_ids=list(range(8)))
    out = np.zeros((B, S, D), np.float32)
    for (b, g), r in zip(metas, res.results):
        out[b] += np.asarray(r["outp"], np.float32).T
    return out


if __name__ == "__main__":
    build_program()
    print("program built ok")


# revision 29
# speedup vs baseline: 1.2127x; 1.0144x over previous
"""Trainium2 Bass kernel for AttentiveSSMNoProjCyc (sparse_attention).

Sharding: 8 cores = 2 batches x 4 head-groups (4 heads / 256 channels each).
Per core, [channel, time] layout, bf16 datapath (tolerance is 2e-2):
  - SSM scans via tensor_tensor_scan (DVE, f32 A-tables from host / bf16 x);
    (1-a)*h scaling on Act, residual add on DVE in bf16
  - RoPE never applied to full sequences; rotation algebra instead:
      band s=t   : q.k unrotated (R(t)^T R(t) = I)
      band s=t-1 : k' = R(-1) k via a constant block-diag PE matmul
      boundary   : score = (q*cos_t).u + (q*sin_t).u~ with u = R(s_b) k_b
                   rotated cheaply on 48 columns
  - head score reduction via 64-block-ones matmul, which also replicates
    scores/denominators across each head's 64 channel partitions
  - Pool (gpsimd) engine: SWDGE input DMAs, mask muls, big bf16 adds
  - all matmuls bf16 (4x PE rate); inputs/outputs bf16 (half DMA traffic)
Host: slice/transpose/pack tables; sum 4 bf16 partials per batch in f32.
"""
import numpy as np
import ml_dtypes

import concourse.bass as bass
import concourse.mybir as mybir
from concourse.bass_utils import run_bass_kernel_spmd
from concourse.tile import TileContext
import concourse.tile as _tile_mod
from concourse.vector_clock import ScopedClock as _ScopedClock


def _split_drain_and_barrier(self, tick_clock, wait_clock):
    """Tail drain, with its sem waits spread over chained SP nops.

    Walrus's TPB_CTRL lowering only accepts a couple of sync waits per
    instruction; redistribute them one-per-nop (same engine, program
    order => semantics preserved).
    """
    probe = self.nc.sync.nop()
    wait_clock.add_sem_waits(
        probe.ins, _ScopedClock({None: tick_clock.global_clock})
    )
    si = probe.ins.sync_info
    waits = list(si.on_wait) if si is not None else []
    upds = list(si.on_update) if si is not None else []
    MAXW = 1
    if len(waits) > MAXW:
        probe.ins.sync_info = mybir.SyncInfo(on_wait=waits[:MAXW],
                                             on_update=upds)
        for i in range(MAXW, len(waits), MAXW):
            extra = self.nc.sync.nop()
            extra.ins.sync_info = mybir.SyncInfo(
                on_wait=waits[i:i + MAXW], on_update=[])
    self.nc.sync.drain()

    self.nc.all_engine_barrier()
    assert self.sems is not None
    popped = self.nc._tile_sem_poison_stack.pop()
    assert popped is self._sem_poison
    self.nc.clear_and_free_semaphores(list(self.sems.allocated().values()))
    self.nc.all_engine_barrier()


_tile_mod.TileContext._drain_and_barrier = _split_drain_and_barrier


def _cap_sync_waits(nc, cap=1):
    """Hoist excess sync waits onto same-engine carrier NOPs (walrus only
    accepts `cap` waits per instruction)."""
    nid = [0]

    def mknop(engine, waits):
        nid[0] += 1
        nop = mybir.InstNoOp(name=f"I-capw-{nid[0]}", ins=[], outs=[])
        nop.engine = engine
        nop.sync_info = mybir.SyncInfo(on_wait=list(waits), on_update=[])
        return nop

    for bb in nc.m.functions[0].blocks:
        il = bb.instructions
        i = 0
        while i < len(il):
            ins = il[i]
            si = ins.sync_info
            nw = len(si.on_wait) if si is not None else 0
            if nw > cap:
                waits = list(si.on_wait)
                ins.sync_info = mybir.SyncInfo(on_wait=waits[:cap],
                                               on_update=list(si.on_update))
                rest = waits[cap:]
                pos = i
                for j in range(0, len(rest), cap):
                    il.insert(pos, mknop(ins.engine, rest[j:j + cap]))
                    pos += 1
                    i += 1
            i += 1


B, S, D, H, HD = 2, 2048, 1024, 16, 64
NG = 4            # head-groups per batch
CH = 256          # channels per core (4 heads)
NB = 48           # padded boundary columns (33 real)
NBD = 112         # blockdiag boundary cols: head0 -> 0:48, head1 -> 64:112
NCHUNK = 4
CS = S // NCHUNK  # 512
F32 = mybir.dt.float32
BF16 = mybir.dt.bfloat16
AL = mybir.AluOpType
AF = mybir.ActivationFunctionType
NEG = -1e30
BF = ml_dtypes.bfloat16


def _boundaries():
    K_, LAYER_, NLAYERS_, MAXLEN_ = 64, 4, 16, 16384
    off = min(K_ - 1, LAYER_ * (K_ // NLAYERS_))
    bl = [b - off for b in range(K_ - 1, MAXLEN_, K_)]
    if bl[-1] != MAXLEN_ - 1:
        bl.append(MAXLEN_ - 1)
    if bl[0] != 0:
        bl.insert(0, 0)
    b = np.asarray(bl)
    b = b[b < S].copy()
    b[-1] = S - 1
    return b


BND = _boundaries()
NBR = len(BND)  # 33

SHUF_XOR1 = [i ^ 1 for i in range(32)]


def build_program():
    nc = bass.Bass()
    dp = nc.declare_dram_parameter
    xt16 = dp("xt16", [D, S], BF16, isOutput=False)
    wqp = dp("wqp", [128, 8 * CH], BF16, isOutput=False)
    wop = dp("wop", [128, 2 * D], BF16, isOutput=False)
    nrs = dp("nrs", [128, S], BF16, isOutput=False)
    ctab = dp("ctab", [128, S], BF16, isOutput=False)
    stab = dp("stab", [128, S], BF16, isOutput=False)
    maskb = dp("maskb", [NBD, S], BF16, isOutput=False)
    scp = dp("scp", [128, 233], F32, isOutput=False)
    bcp = dp("bcp", [128, 512], BF16, isOutput=False)
    outp = dp("outp", [D, S], BF16, isOutput=True)

    with TileContext(nc) as tc:
        with (
            tc.tile_pool(name="persist", bufs=1) as pp,
            tc.tile_pool(name="xbig", bufs=1) as xb,       # 8 x-tiles resident
            tc.tile_pool(name="sc8k", bufs=2) as sc,       # f32 h scratch
            tc.tile_pool(name="hs16", bufs=2) as hsp,      # bf16 scaled h
            tc.tile_pool(name="px", bufs=4) as px,         # xq,kp -> n1,n2,stage
            tc.tile_pool(name="eat", bufs=4) as ea,        # e1,e0 -> attn
            tc.tile_pool(name="mid", bufs=2) as md,        # s12 (+denE tag)
            tc.tile_pool(name="mid2", bufs=2) as md2,      # den -> s3
            tc.tile_pool(name="small", bufs=1) as ck,
            tc.tile_pool(name="psA", bufs=5, space="PSUM") as psA,
            tc.tile_pool(name="psB", bufs=3, space="PSUM") as psB,
            nc.allow_low_precision(reason="bf16 datapath; tol 2e-2"),
        ):
            # ============ input DMAs ============
            # Critical path via SP HWDGE: scp, x0, Ak0, x1, Ak1, Av0, Av1,
            # x2, x3.  Bulk via Pool SWDGE (keeps HWDGE short at startup).
            scp_t = pp.tile([128, 233], F32, tag="scp", name="scp_t")
            nc.sync.dma_start(out=scp_t, in_=scp[:, :])
            x_t = [xb.tile([128, S], BF16, tag=f"xt{k}", name=f"x_t{k}")
                   for k in range(8)]
            nrs_t = pp.tile([128, S], BF16, tag="nrs", name="nrs_t")
            nc.sync.dma_start(out=nrs_t, in_=nrs[:, :])
            nc.sync.dma_start(out=x_t[0], in_=xt16[0:128, :])
            nc.sync.dma_start(out=x_t[1], in_=xt16[128:256, :])
            nc.sync.dma_start(out=x_t[2], in_=xt16[256:384, :])
            nc.sync.dma_start(out=x_t[3], in_=xt16[384:512, :])
            bcp_t = pp.tile([128, 512], BF16, tag="bcp", name="bcp_t")
            ctab_t = pp.tile([128, S], BF16, tag="ctab", name="ctab_t")
            stab_t = pp.tile([128, S], BF16, tag="stab", name="stab_t")
            maskb_t = pp.tile([NBD, S], BF16, tag="maskb", name="maskb_t")
            wqp_t = px.tile([128, 8 * CH], BF16, tag="px", name="wqp_t")
            wop_t = pp.tile([128, 2 * D], BF16, tag="wop", name="wop_t")
            nc.gpsimd.dma_start(out=wqp_t, in_=wqp[:, :])
            for k in range(4, 8):
                nc.gpsimd.dma_start(out=x_t[k],
                                    in_=xt16[k * 128:(k + 1) * 128, :])
            # A_v tables built on Pool while Act handles A_k + h-scales
            av_t = [sc.tile([128, S], F32, tag="avtile", name=f"Av{dt}")
                    for dt in range(2)]
            for dt in range(2):
                nc.gpsimd.tensor_scalar(out=av_t[dt], in0=nrs_t,
                                        scalar1=scp_t[:, 103 + dt:104 + dt],
                                        scalar2=None, op0=AL.mult)
            nc.gpsimd.dma_start(out=ctab_t, in_=ctab[:, :])
            nc.gpsimd.dma_start(out=stab_t, in_=stab[:, :])
            nc.gpsimd.dma_start(out=bcp_t, in_=bcp[:, :])
            nc.gpsimd.dma_start(out=maskb_t, in_=maskb[:, :])
            nc.gpsimd.dma_start(out=wop_t, in_=wop[:, :])

            cbb = scp_t[:, 0:48]
            sbb = scp_t[:, 48:96]
            pmv = scp_t[:, 96:97]
            omap = scp_t[:, 97:101]       # 1 - sigmoid(a), host-computed
            asig = scp_t[:, 101:105]      # sigmoid(a), host-computed
            ident32 = scp_t[:, 105:233]
            O128 = bcp_t[:, 0:128]
            obv128 = bcp_t[:, 128:256]
            rotm = bcp_t[:, 256:384]

            # ============ PE p-state warmup ============
            # The cost model prices matmuls by how long the PE has been
            # continuously busy (3us to full speed).  Burn the ramp on
            # dummy f32 matmuls over scp while the x tiles stream in.
            warm = psA.tile([128, CS], F32, tag="psa", name="warm")
            for i in range(5):
                nc.tensor.matmul(warm[:, 0:233], scp_t[:, 0:128], scp_t,
                                 start=(i == 0), stop=(i == 4))
            wsink = ck.tile([128, 8], F32, tag="wsink", name="wsink")
            nc.scalar.activation(wsink, warm[:, 0:8], AF.Copy)

            # ============ SSM scans (k then v) ============
            kpre = [pp.tile([128, S], BF16, tag=f"kpre{dt}", name=f"kpre{dt}")
                    for dt in range(2)]
            v16 = [pp.tile([128, S], BF16, tag=f"v16{dt}", name=f"v16{dt}")
                   for dt in range(2)]
            SEG = 1072   # reset-aligned split (48 + 64*16)
            ats = {}
            for dt in range(2):
                A_t = sc.tile([128, S], F32, tag="atile", name=f"A{dt}")
                nc.scalar.activation(A_t, nrs_t, AF.Copy,
                                     scale=asig[:, dt:dt + 1])
                ats[dt] = A_t
            # interleave: all half-0 sub-scans first (unblocks chunks 0/1
            # downstream), then half-1.  h and hs scratch per (chain, half).
            for half, (c0, c1) in enumerate(((0, SEG), (SEG, S))):
                sl = slice(c0, c1)
                w = c1 - c0
                for par, outs in enumerate((kpre, v16)):
                    for dt in range(2):
                        col = 2 * par + dt
                        A_t = ats[dt] if par == 0 else av_t[dt]
                        h_t = sc.tile([128, SEG], F32, tag="sc8k",
                                      name=f"h{col}_{half}")
                        nc.vector.tensor_tensor_scan(
                            out=h_t[:, 0:w], data0=A_t[:, sl],
                            data1=x_t[dt][:, sl], initial=0.0,
                            op0=AL.mult, op1=AL.add)
                        hst = hsp.tile([128, SEG], BF16, tag="hs16",
                                       name=f"hs{col}_{half}")
                        nc.scalar.activation(hst[:, 0:w], h_t[:, 0:w],
                                             AF.Copy,
                                             scale=omap[:, col:col + 1])
                        nc.vector.tensor_add(out=outs[dt][:, sl],
                                             in0=hst[:, 0:w],
                                             in1=x_t[dt][:, sl])

            # ============ Q projection (stationary-reuse order) ============
            xq = [px.tile([128, S], BF16, tag="px", name=f"xq{dt}")
                  for dt in range(2)]
            for m in range(2):
                accs = [psA.tile([128, CS], F32, tag="psa",
                                 name=f"qacc{m}_{c}") for c in range(NCHUNK)]
                for k in range(8):
                    st_sl = wqp_t[:, k * CH + m * 128:k * CH + m * 128 + 128]
                    for c in range(NCHUNK):
                        nc.tensor.matmul(
                            accs[c], st_sl,
                            x_t[k][:, c * CS:(c + 1) * CS],
                            start=(k == 0), stop=(k == 7))
                for c in range(NCHUNK):
                    nc.scalar.activation(xq[m][:, c * CS:(c + 1) * CS],
                                         accs[c], AF.Copy)

            # ============ k' = R(-1) k via PE ============
            kp = [px.tile([128, S], BF16, tag="px", name=f"kp{dt}")
                  for dt in range(2)]
            for dt in range(2):
                for c in range(NCHUNK):
                    chs = slice(c * CS, (c + 1) * CS)
                    kps = psB.tile([128, CS], F32, tag="psb",
                                   name=f"kps{dt}_{c}")
                    nc.tensor.matmul(kps, rotm, kpre[dt][:, chs],
                                     start=True, stop=True)
                    nc.scalar.activation(kp[dt][:, chs], kps, AF.Copy)

            # ============ qc/qs + band products ============
            # qc/qs reuse x-tile buffers (x_t[4..7] dead after Q-proj)
            qc = [xb.tile([128, S], BF16, tag=f"xt{4 + dt}", name=f"qc{dt}")
                  for dt in range(2)]
            qs = [xb.tile([128, S], BF16, tag=f"xt{6 + dt}", name=f"qs{dt}")
                  for dt in range(2)]
            prod1 = [sc.tile([128, S], BF16, tag="pr16", name=f"pr1_{dt}")
                     for dt in range(2)]
            prod0 = [sc.tile([128, S], BF16, tag="pr16", name=f"pr0_{dt}")
                     for dt in range(2)]
            nc.gpsimd.tensor_tensor(out=qc[0], in0=xq[0], in1=ctab_t,
                                    op=AL.mult)
            nc.gpsimd.tensor_tensor(out=qs[0], in0=xq[0], in1=stab_t,
                                    op=AL.mult)
            for dt in range(2):
                nc.vector.tensor_mul(out=prod1[dt], in0=xq[dt],
                                     in1=kpre[dt])
                nc.vector.memset(prod0[dt][:, 0:1], 0.0)
                nc.vector.tensor_mul(out=prod0[dt][:, 1:S],
                                     in0=xq[dt][:, 1:S],
                                     in1=kp[dt][:, 0:S - 1])
            nc.vector.tensor_mul(out=qc[1], in0=xq[1], in1=ctab_t)
            nc.vector.tensor_mul(out=qs[1], in0=xq[1], in1=stab_t)

            # band scores (PE + Act)
            e1 = [ea.tile([128, S], BF16, tag="eat", name=f"e1_{dt}")
                  for dt in range(2)]
            e0 = [ea.tile([128, S], BF16, tag="eat", name=f"e0_{dt}")
                  for dt in range(2)]
            for dt in range(2):
                for c in range(NCHUNK):
                    chs = slice(c * CS, (c + 1) * CS)
                    s1p = psB.tile([128, CS], F32, tag="psb",
                                   name=f"s1p{dt}_{c}")
                    nc.tensor.matmul(s1p, O128, prod1[dt][:, chs],
                                     start=True, stop=True)
                    nc.scalar.activation(e1[dt][:, chs], s1p, AF.Exp,
                                         scale=0.125)
                    s0p = psB.tile([128, CS], F32, tag="psb",
                                   name=f"s0p{dt}_{c}")
                    nc.tensor.matmul(s0p, O128, prod0[dt][:, chs],
                                     start=True, stop=True)
                    if c == 0:
                        nc.vector.memset(s0p[:, 0:1], NEG)
                    nc.scalar.activation(e0[dt][:, chs], s0p, AF.Exp,
                                         scale=0.125)

            # ============ boundary keys ============
            embd = [pp.tile([128, S], BF16, tag=f"embd{dt}", name=f"embd{dt}")
                    for dt in range(2)]
            vbT = [pp.tile([128, 64], BF16, tag=f"vbT{dt}", name=f"vbT{dt}")
                   for dt in range(2)]
            for dt in range(2):
                kb = ck.tile([128, NB], BF16, tag="kb", name=f"kb{dt}")
                vb = ck.tile([128, NB], F32, tag="vb", name=f"vb{dt}")
                for src_t, dst_t in ((kpre[dt], kb), (v16[dt], vb)):
                    nc.vector.tensor_copy(out=dst_t[:, 0:1],
                                          in_=src_t[:, 0:1])
                    nc.vector.tensor_copy(
                        out=dst_t[:, 1:32],
                        in_=src_t.rearrange("p (a b) -> p a b",
                                            b=64)[:, 0:31, 47])
                    nc.vector.tensor_copy(out=dst_t[:, 32:33],
                                          in_=src_t[:, S - 1:S])
                    nc.vector.memset(dst_t[:, 33:NB], 0.0)
                # u = R(s_b) k_b : pair rotation with per-col coeffs
                kbsh = ck.tile([128, NB], BF16, tag="kbs", name=f"kbsh{dt}")
                nc.vector.stream_shuffle(kbsh, kb, SHUF_XOR1)
                t1 = ck.tile([128, NB], BF16, tag="kbt", name=f"t1_{dt}")
                nc.vector.tensor_mul(out=t1, in0=kb, in1=cbb)
                t2 = ck.tile([128, NB], BF16, tag="kbu", name=f"t2_{dt}")
                nc.vector.tensor_mul(out=t2, in0=kbsh, in1=sbb)
                u16 = ck.tile([128, NB], BF16, tag="kbv", name=f"u16_{dt}")
                nc.vector.tensor_add(out=u16, in0=t1, in1=t2)
                ush = ck.tile([128, NB], BF16, tag="kbw", name=f"ush{dt}")
                nc.vector.stream_shuffle(ush, u16, SHUF_XOR1)
                usw = ck.tile([128, NB], BF16, tag="kbx", name=f"usw{dt}")
                nc.vector.tensor_scalar(out=usw, in0=ush, scalar1=pmv,
                                        scalar2=None, op0=AL.mult)
                kbdA = ck.tile([128, NBD], BF16, tag="kbdA", name=f"kbdA{dt}")
                kbdB = ck.tile([128, NBD], BF16, tag="kbdB", name=f"kbdB{dt}")
                for src_t, dst_t in ((u16, kbdA), (usw, kbdB)):
                    nc.vector.memset(dst_t, 0.0)
                    nc.vector.tensor_copy(out=dst_t[0:64, 0:48],
                                          in_=src_t[0:64, :])
                    nc.vector.tensor_copy(out=dst_t[64:128, 64:112],
                                          in_=src_t[64:128, :])
                # vbT: transpose boundary values
                for hh in range(2):
                    tp = psB.tile([128, CS], F32, tag="psb",
                                  name=f"tp{dt}_{hh}")
                    nc.tensor.transpose(tp[0:48, 0:64],
                                        vb[hh * 64:(hh + 1) * 64, 0:48],
                                        ident32[hh * 64:(hh + 1) * 64,
                                                hh * 64:(hh + 1) * 64],
                                        tile_position=(hh * 64, 0))
                    nc.scalar.activation(vbT[dt][hh * 64:hh * 64 + 48, :],
                                         tp[0:48, 0:64], AF.Copy)
                for c in range(NCHUNK):
                    chs = slice(c * CS, (c + 1) * CS)
                    eb = psB.tile([128, CS], F32, tag="psb",
                                  name=f"eb{dt}_{c}")
                    nc.tensor.matmul(eb[0:NBD, :], kbdA, qc[dt][:, chs],
                                     start=True, stop=False)
                    nc.tensor.matmul(eb[0:NBD, :], kbdB, qs[dt][:, chs],
                                     start=False, stop=True)
                    nc.scalar.activation(embd[dt][0:NBD, chs], eb[0:NBD, :],
                                         AF.Exp, scale=0.125)
                    # mask multiply on Pool (bf16, SBUF-only)
                    nc.gpsimd.tensor_tensor(out=embd[dt][0:NBD, chs],
                                            in0=embd[dt][0:NBD, chs],
                                            in1=maskb_t[:, chs], op=AL.mult)

            # ============ n1/n2 early (feed Pool s12) ============
            n1 = [px.tile([128, S], BF16, tag="px", name=f"n1_{dt}")
                  for dt in range(2)]
            n2 = [px.tile([128, S], BF16, tag="px", name=f"n2_{dt}")
                  for dt in range(2)]
            s12 = [md.tile([128, S], BF16, tag="mid", name=f"s12_{dt}")
                   for dt in range(2)]
            for half in range(2):
                lo, hi = half * 2 * CS, (half + 1) * 2 * CS
                for dt in range(2):
                    nc.vector.tensor_mul(out=n1[dt][:, lo:hi],
                                         in0=e1[dt][:, lo:hi],
                                         in1=v16[dt][:, lo:hi])
                    if half == 0:
                        nc.vector.memset(n2[dt][:, 0:1], 0.0)
                    nc.vector.tensor_mul(
                        out=n2[dt][:, max(lo, 1):hi],
                        in0=e0[dt][:, max(lo, 1):hi],
                        in1=v16[dt][:, max(lo, 1) - 1:hi - 1])
                    nc.vector.tensor_add(out=s12[dt][:, lo:hi],
                                         in0=n1[dt][:, lo:hi],
                                         in1=n2[dt][:, lo:hi])

            # ============ denominators ============
            rd = [pp.tile([128, S], BF16, tag=f"rd{dt}", name=f"rd{dt}")
                  for dt in range(2)]
            dens = [md2.tile([128, S], BF16, tag="mid2", name=f"den{dt}")
                    for dt in range(2)]
            denEs = [md.tile([128, S], BF16, tag="midE", name=f"denE{dt}",
                             bufs=2) for dt in range(2)]
            for half in range(2):
                hsl = slice(half * 2 * CS, (half + 1) * 2 * CS)
                for dt in range(2):
                    nc.vector.tensor_add(out=denEs[dt][:, hsl],
                                         in0=e1[dt][:, hsl],
                                         in1=e0[dt][:, hsl])
                    for c in (2 * half, 2 * half + 1):
                        chs = slice(c * CS, (c + 1) * CS)
                        bs = psB.tile([128, CS], F32, tag="psb",
                                      name=f"bs{dt}_{c}")
                        nc.tensor.matmul(bs, obv128[0:NBD, :],
                                         embd[dt][0:NBD, chs],
                                         start=True, stop=True)
                        nc.vector.tensor_add(out=dens[dt][:, chs],
                                             in0=denEs[dt][:, chs], in1=bs)
                    nc.vector.reciprocal(rd[dt][:, hsl], dens[dt][:, hsl])

            # ============ PV + combine ============
            attn = [ea.tile([128, S], BF16, tag="eat", name=f"attn{dt}")
                    for dt in range(2)]
            s3 = [md2.tile([128, S], BF16, tag="mid2", name=f"s3_{dt}")
                  for dt in range(2)]
            for c in range(NCHUNK):
                chs = slice(c * CS, (c + 1) * CS)
                for dt in range(2):
                    pv = psA.tile([128, CS], F32, tag="psa",
                                  name=f"pv{dt}_{c}")
                    for hh in range(2):
                        nc.tensor.matmul(
                            pv[hh * 64:(hh + 1) * 64, :],
                            vbT[dt][hh * 64:hh * 64 + 48, :],
                            embd[dt][hh * 64:hh * 64 + 48, chs],
                            start=True, stop=True,
                            tile_position=(hh * 64, hh * 64))
                    nc.vector.tensor_add(out=s3[dt][:, chs],
                                         in0=s12[dt][:, chs], in1=pv)
                    nc.vector.tensor_mul(out=attn[dt][:, chs],
                                         in0=s3[dt][:, chs],
                                         in1=rd[dt][:, chs])

            # ============ output projection ============
            for m in range(8):
                stage = px.tile([128, S], BF16, tag="px", name=f"stage{m}")
                accs = [psA.tile([128, CS], F32, tag="psa",
                                 name=f"oacc{m}_{c}") for c in range(NCHUNK)]
                for c in range(NCHUNK):
                    chs = slice(c * CS, (c + 1) * CS)
                    for k in range(2):
                        st_sl = wop_t[:,
                                      k * D + m * 128:k * D + (m + 1) * 128]
                        nc.tensor.matmul(accs[c], st_sl, attn[k][:, chs],
                                         start=(k == 0), stop=(k == 1))
                    if c % 2 == 1:
                        nc.vector.tensor_copy(out=stage[:, chs], in_=accs[c])
                    else:
                        nc.scalar.activation(stage[:, chs], accs[c], AF.Copy)
                nc.sync.dma_start(out=outp[m * 128:(m + 1) * 128, :],
                                  in_=stage)
    _cap_sync_waits(nc)
    return nc


# ---------------- host side ----------------

def _host_consts(fc):
    # per-partition pair index: i(p) = (p % 64) // 2
    pidx = (np.arange(128) % 64) // 2
    cos_t = fc[:, :, 0, 0]          # (S, 32)
    sin_t = fc[:, :, 1, 0]          # (S, 32)
    ctab = np.ascontiguousarray(cos_t[:, pidx].T).astype(BF)   # (128, S)
    stabt = np.ascontiguousarray(sin_t[:, pidx].T).astype(BF)

    nrs1 = np.ones(S, np.float32)
    starts = np.concatenate([[0], BND[:-1] + 1])
    nrs1[starts] = 0.0

    mb = np.zeros((NBD, S), np.float32)
    t = np.arange(S)
    for hh in range(2):
        for jb in range(NBR):
            mb[hh * 64 + jb] = (t >= BND[jb] + 2).astype(np.float32)
    mb16 = mb.astype(BF)

    cbb = np.zeros((128, NB), np.float32)
    sbb = np.zeros((128, NB), np.float32)
    even = (np.arange(128) % 2 == 0)
    for jb in range(NBR):
        cb = cos_t[BND[jb]][pidx]          # (128,)
        sb = sin_t[BND[jb]][pidx]
        cbb[:, jb] = cb
        sbb[:, jb] = np.where(even, -sb, sb)
    pm = np.where(even, 1.0, -1.0).astype(np.float32)

    # bcp pack: O128 | obv128 | rotm | spare  (bf16)
    O128 = np.zeros((128, 128), np.float32)
    O128[0:64, 0:64] = 1.0
    O128[64:128, 64:128] = 1.0
    obv = np.zeros((128, 128), np.float32)
    obv[0:48, 0:64] = 1.0
    obv[64:112, 64:128] = 1.0
    rotm = np.zeros((128, 128), np.float32)
    c1 = cos_t[1][pidx]   # (128,) per-partition cos(theta_i)
    s1 = sin_t[1][pidx]
    for j in range(64):
        pe_, po = 2 * j, 2 * j + 1
        # kp[2i] = c1*ke + s1*ko ; kp[2i+1] = -s1*ke + c1*ko
        rotm[pe_, pe_] = c1[pe_]
        rotm[po, pe_] = s1[pe_]
        rotm[pe_, po] = -s1[pe_]
        rotm[po, po] = c1[pe_]
    bcp = np.concatenate([O128, obv, rotm,
                          np.zeros((128, 128), np.float32)],
                         axis=1).astype(BF)

    return ctab, stabt, nrs1, mb16, cbb, sbb, pm, bcp


_prog = None


def make_in_maps(x, fc, wq_, wo_, a_k_, a_v_):
    ctab, stabt, nrs1, mb16, cbb, sbb, pm, bcp = _host_consts(fc)
    x16 = x.astype(BF)
    ident = np.eye(128, dtype=np.float32)
    in_maps, metas = [], []
    for b in range(B):
        xT = np.ascontiguousarray(x16[b].T)
        for g in range(NG):
            c0 = g * CH
            perm = np.concatenate([np.arange(c0, c0 + CH),
                                   np.arange(0, c0),
                                   np.arange(c0 + CH, D)]).astype(np.int64)
            xt_core = np.ascontiguousarray(xT[perm])
            wqt = wq_[c0:c0 + CH, :].T[perm]            # (1024, 256)
            wqpk = np.ascontiguousarray(
                wqt.reshape(8, 128, CH).transpose(1, 0, 2).reshape(
                    128, 8 * CH)).astype(BF)
            wot = wo_[:, c0:c0 + CH].T                  # (256, 1024)
            wopk = np.ascontiguousarray(
                wot.reshape(2, 128, D).transpose(1, 0, 2).reshape(
                    128, 2 * D)).astype(BF)
            a_k = 1.0 / (1.0 + np.exp(-a_k_[c0:c0 + CH]))   # sigmoid
            a_v = 1.0 / (1.0 + np.exp(-a_v_[c0:c0 + CH]))
            omap = np.stack([1.0 - a_k[0:128], 1.0 - a_k[128:256],
                             1.0 - a_v[0:128], 1.0 - a_v[128:256]],
                            axis=1).astype(np.float32)
            asig = np.stack([a_k[0:128], a_k[128:256],
                             a_v[0:128], a_v[128:256]],
                            axis=1).astype(np.float32)
            scpk = np.concatenate([cbb, sbb, pm[:, None], omap, asig,
                                   ident], axis=1).astype(np.float32)
            in_maps.append({
                "xt16": xt_core, "wqp": wqpk, "wop": wopk,
                "nrs": np.broadcast_to(nrs1, (128, S)).astype(BF).copy(),
                "ctab": ctab, "stab": stabt, "maskb": mb16, "scp": scpk,
                "bcp": bcp,
            })
            metas.append((b, g))
    return in_maps, metas


def kernel(x, freq_cis, wq, wo, a_k, a_v):
    global _prog
    x = np.asarray(x, np.float32)
    fc = np.asarray(freq_cis, np.float32)
    wq_ = np.asarray(wq, np.float32)
    wo_ = np.asarray(wo, np.float32)
    a_k_ = np.asarray(a_k, np.float32)
    a_v_ = np.asarray(a_v, np.float32)
    in_maps, metas = make_in_maps(x, fc, wq_, wo_, a_k_, a_v_)
    if _prog is None:
        _prog = build_program()
    res = run_bass_kernel_spmd(_prog, in_maps, core_ids=list(range(8)))
    out = np.zeros((B, S, D), np.float32)
    for (b, g), r in zip(metas, res.results):
        out[b] += np.asarray(r["outp"], np.float32).T
    return out


if __name__ == "__main__":
    build_program()
    print("program built ok")
